# revision 1
# baseline (speedup 1.0000x reference)
"""Causal self-attention (B=8, T=1024, C=768, H=12, D=64) on 8 TRN2 NeuronCores.

Sharding: data-parallel over batch — core b handles batch element b. No
collectives. Host pre-transposes x to x^T[b] and pre-casts operands to bf16;
all matmuls run bf16 with fp32 PSUM accumulation.

Per-core algorithm:
  v = x Wv in [t, c] layout (x^T stationary); v bias folded into the y^T
  stage (exact: softmax rows sum to 1). q^T,k^T = (Wqkv^T x^T + b) in
  [c3, t] layout (weights stationary). Per head h, key-block j (128 keys):
  S^T = K_j Q^T in PSUM [keys, q] (causal: only q >= 128j columns; blocks
  pack into [128,1024] PSUM tiles as {j0},{j1,j7},{j2,j6},{j3,j5},{j4} so
  one ACT exp covers each tile, scale=1/8), triangular mask-multiply on
  diagonal 128x128 blocks. Per q-tile i: y'[q,65] = sum_j P_j^T.T @ [V_j|1]
  accumulated in PSUM (two heads x two i-steps share one PSUM bank); col 64
  is the softmax denominator. Normalize via per-partition reciprocal+scale,
  PE-transpose into a per-pair [128, 1024] bf16 PSUM strip (head parity in
  partition halves), one DVE pass per pair adds the v-bias and lands y^T in
  SBUF. out[t, c] = y^T.T @ Wproj + b_proj (bias via K=1 ones matmul).

Emission is software-pipelined (static per-engine order => head-of-line
blocking): cycle hp interleaves AV(hp) i-steps with qk(hp+1) half-chunks
and S(hp+1) groups so PE fills ACT-paced exp stalls; v tiles fill the S(0)
cold start.

PSUM budget (8 banks): big [128,1024]fp32 x2 (4, shared v/qk/S/o) +
y' [128,512]fp32 x2 (2, two heads x two i-steps packed) +
tr [128,1024]bf16 x2 (2).
"""

import numpy as np
import ml_dtypes

B, T, C = 8, 1024, 768
H, D = 12, 64
C3 = 3 * C
KC = C // 128          # 6 contraction chunks over c_in
TT = T // 128          # 8 t-tiles of 128
NPAIR = H // 2

BIG_BUFS = 3
SM_BUFS = 1
TR_BUFS = 1
PP_BUFS = 20           # 10 P segs live per pair (5 groups x 2 heads)

_BF16 = ml_dtypes.bfloat16

_compiled = {}


def _build():
    from concourse import bacc, mybir
    import concourse.tile as tile
    from concourse.masks import make_identity, make_upper_triangular

    fp32 = mybir.dt.float32
    bf16 = mybir.dt.bfloat16

    nc = bacc.Bacc("TRN2", target_bir_lowering=False, debug=False,
                   enable_asserts=True, num_devices=B)

    xT = nc.dram_tensor("xT", [C, T], bf16, kind="ExternalInput")
    wqkv = nc.dram_tensor("wqkv", [C, C3], bf16, kind="ExternalInput")
    wproj = nc.dram_tensor("wproj", [C, C], bf16, kind="ExternalInput")
    # b_qkv rearranged host-side to [128, 18]: col j holds b_qkv[128j:128j+128]
    bqkv = nc.dram_tensor("bqkv", [128, C3 // 128], fp32, kind="ExternalInput")
    # b_proj rearranged host-side to [128, 6]: col j holds b_proj[128j:128j+128]
    bproj = nc.dram_tensor("bproj", [128, C // 128], fp32, kind="ExternalInput")
    # out is produced transposed [C, T] in bf16; host transposes + casts back
    out = nc.dram_tensor("out", [C, T], bf16, kind="ExternalOutput")

    Exp = mybir.ActivationFunctionType.Exp
    # S-block packing: groups of (j, base column) sharing one [128,1024]
    # PSUM tile => one exp per tile. Bases keep each block inside the tile.
    GROUPS = [((4, 0),), ((3, 0), (5, 640)), ((2, 0), (6, 768)),
              ((1, 0), (7, 896)), ((0, 0),)]

    with tile.TileContext(nc) as tc:
        with (
            tc.tile_pool(name="const", bufs=1) as const,
            tc.tile_pool(name="pP", bufs=PP_BUFS) as pP,
            tc.tile_pool(name="small", bufs=6) as small,
            tc.tile_pool(name="osb", bufs=4) as osb,
            tc.tile_pool(name="ps_big", bufs=BIG_BUFS, space="PSUM") as ps_big,
            tc.tile_pool(name="ps_sm", bufs=SM_BUFS, space="PSUM") as ps_sm,
            tc.tile_pool(name="ps_tr", bufs=TR_BUFS, space="PSUM") as ps_tr,
        ):
            # ---- persistent SBUF loads ----
            # All input streaming via the two HWDGE queues (SP + ACT), in
            # need-order: hp0+1 q/k sliver cols + x^T first (qk(0)), then
            # v cols, hp2-5 q/k cols, wproj.  DMA_ENGINES is a serial
            # resource: issue order IS the arrival order.
            bqkv_sb = const.tile([128, C3 // 128], fp32, tag="bqkv", name="bqkv")
            wq_big = const.tile([128, KC, C3], bf16, tag="wqkv", name="wqkv")
            wqkv_sb = [wq_big[:, kc] for kc in range(KC)]
            wqkv_src = wqkv.rearrange("(k p) c -> p k c", k=KC)
            xT_sb = [const.tile([128, T], bf16, tag=f"xT{kc}", name=f"xT{kc}")
                     for kc in range(KC)]
            nc.sync.dma_start(wq_big[:, :, 0:256], wqkv_src[:, :, 0:256])
            nc.scalar.dma_start(xT_sb[0][:], xT[0:128, :])
            nc.sync.dma_start(wq_big[:, :, C:C + 256], wqkv_src[:, :, C:C + 256])
            nc.scalar.dma_start(xT_sb[1][:], xT[128:256, :])
            nc.sync.dma_start(xT_sb[2][:], xT[256:384, :])
            nc.scalar.dma_start(xT_sb[3][:], xT[384:512, :])
            nc.sync.dma_start(xT_sb[4][:], xT[512:640, :])
            nc.scalar.dma_start(xT_sb[5][:], xT[640:768, :])
            nc.sync.dma_start(bqkv_sb[:], bqkv[:, :])
            nc.sync.dma_start(wq_big[:, :, 2 * C:], wqkv_src[:, :, 2 * C:])
            nc.sync.dma_start(wq_big[:, :, 256:C], wqkv_src[:, :, 256:C])
            nc.sync.dma_start(wq_big[:, :, C + 256:2 * C],
                              wqkv_src[:, :, C + 256:2 * C])
            wproj_big = const.tile([128, KC, C], bf16, tag="wproj", name="wproj")
            wproj_sb = [wproj_big[:, kc] for kc in range(KC)]
            nc.sync.dma_start(
                wproj_big[:],
                wproj.rearrange("(k p) c -> p k c", k=KC),
            )
            bproj_sb = const.tile([128, C // 128], fp32, tag="bproj", name="bproj")
            nc.sync.dma_start(bproj_sb[:], bproj[:, :])
            ident_sb = const.tile([128, 128], bf16, tag="ident", name="ident")
            make_identity(nc, ident_sb[:])
            # keep columns m >= l (query >= key) on the diagonal block
            trimask_sb = const.tile([128, 128], bf16, tag="trimask", name="trimask")
            make_upper_triangular(nc, trimask_sb[:], val=1.0, diag=True)

            qkT_sb = [const.tile([128, T], bf16, tag=f"qkT{c3}", name=f"qkT{c3}")
                      for c3 in range(2 * KC)]
            # v packed [t, 12 heads x (64 + ones col)]
            v_sb = [const.tile([128, H, D + 1], bf16, tag=f"v{tt}", name=f"v{tt}")
                    for tt in range(TT)]
            yT_sb = [const.tile([128, T], bf16, tag=f"yT{kc}", name=f"yT{kc}")
                     for kc in range(KC)]

            def emit_v(tt):
                ps = ps_big.tile([128, 1024], fp32, tag="big", name="v_ps")
                for kc in range(KC):
                    nc.tensor.matmul(
                        ps[:, 0:512],
                        xT_sb[kc][:, tt * 128:(tt + 1) * 128],
                        wqkv_sb[kc][:, 2 * C:2 * C + 512],
                        start=(kc == 0), stop=(kc == KC - 1),
                    )
                    nc.tensor.matmul(
                        ps[:, 512:768],
                        xT_sb[kc][:, tt * 128:(tt + 1) * 128],
                        wqkv_sb[kc][:, 2 * C + 512:3 * C],
                        start=(kc == 0), stop=(kc == KC - 1),
                    )
                vv = v_sb[tt]
                nc.vector.tensor_copy(
                    vv[:, :, 0:D],
                    ps[:, 0:768].rearrange("p (h d) -> p h d", d=D),
                )
                nc.vector.memset(vv[:, :, D:D + 1], 1.0)

            Identity = mybir.ActivationFunctionType.Identity

            def emit_qk_half(hp, which, tchunk, on_act=False):
                c3 = hp if which == "q" else KC + hp
                ps = ps_big.tile([128, 1024], fp32, tag="big", name="qk_ps")
                sl = slice(tchunk * 512, (tchunk + 1) * 512)
                for kc in range(KC):
                    nc.tensor.matmul(
                        ps[:, sl],
                        wqkv_sb[kc][:, c3 * 128:(c3 + 1) * 128],
                        xT_sb[kc][:, sl],
                        start=(kc == 0), stop=(kc == KC - 1),
                    )
                if on_act:
                    nc.scalar.activation(qkT_sb[c3][:, sl], ps[:, sl],
                                         Identity, bias=bqkv_sb[:, c3:c3 + 1])
                else:
                    nc.vector.tensor_scalar_add(
                        qkT_sb[c3][:, sl], ps[:, sl], bqkv_sb[:, c3:c3 + 1],
                    )

            def emit_S_group(hp, segs, grp):
                qT = qkT_sb[hp]
                kT = qkT_sb[KC + hp]
                for h in (2 * hp, 2 * hp + 1):
                    poff = 64 * (h % 2)
                    S = ps_big.tile([128, 1024], fp32, tag="big", name="S")
                    span = 0
                    for j, base in grp:
                        qs = 128 * j
                        w = T - qs
                        span = base + w
                        first = base + min(512 - base % 512, w) if base < 512 \
                            else base + w
                        for a, b_ in ((base, first), (first, base + w)):
                            if b_ <= a:
                                continue
                            nc.tensor.matmul(
                                S[:, a:b_],
                                kT[poff:poff + 64, qs:qs + 128],
                                qT[poff:poff + 64, qs + (a - base):qs + (b_ - base)],
                                start=True, stop=True,
                            )
                    P = pP.tile([128, 1024], bf16, tag="P", name="P")
                    nc.scalar.activation(P[:, 0:span], S[:, 0:span], Exp,
                                         scale=0.125)
                    for j, base in grp:
                        nc.gpsimd.tensor_mul(P[:, base:base + 128],
                                             P[:, base:base + 128],
                                             trimask_sb[:])
                        segs[h][j] = (P, base)

            def emit_S_j4_pair(hp, segs):
                # both heads' j4 block (512 cols each) share one PSUM tile
                # and one exp: halves the ACT op overhead for this group
                qT = qkT_sb[hp]
                kT = qkT_sb[KC + hp]
                S = ps_big.tile([128, 1024], fp32, tag="big", name="S")
                for idx, h in enumerate((2 * hp, 2 * hp + 1)):
                    poff = 64 * (h % 2)
                    nc.tensor.matmul(
                        S[:, 512 * idx:512 * idx + 512],
                        kT[poff:poff + 64, 512:640],
                        qT[poff:poff + 64, 512:1024],
                        start=True, stop=True,
                    )
                P = pP.tile([128, 1024], bf16, tag="P", name="P")
                nc.scalar.activation(P[:], S[:], Exp, scale=0.125)
                for idx, h in enumerate((2 * hp, 2 * hp + 1)):
                    base = 512 * idx
                    nc.gpsimd.tensor_mul(P[:, base:base + 128],
                                         P[:, base:base + 128],
                                         trimask_sb[:])
                    segs[h][4] = (P, base)

            def emit_AV_half(hp, segs, yns, i, y2, half):
                pair = (2 * hp, 2 * hp + 1)
                b0 = 256 * half
                for idx, h in enumerate(pair):
                    c0 = b0 + 128 * idx
                    for j in range(i + 1):
                        P, base = segs[h][j]
                        off = base + 128 * (i - j)
                        nc.tensor.matmul(
                            y2[:, c0:c0 + D + 1],
                            P[:, off:off + 128],
                            v_sb[j][:, h, :],
                            start=(j == 0), stop=(j == i),
                        )
                recip = small.tile([128, 2], fp32, tag="recip", name="recip")
                nc.vector.reciprocal(
                    recip[:],
                    y2[:].rearrange("p (g c) -> p g c", c=128)[:, 2 * half:2 * half + 2, D],
                )
                # both heads' normalized y packed [128, 128] -> one transpose
                yn = small.tile([128, 2 * D], bf16, tag="yn", name="yn",
                                bufs=10)
                for idx, h in enumerate(pair):
                    c0 = b0 + 128 * idx
                    nc.vector.tensor_scalar_mul(yn[:, idx * D:(idx + 1) * D],
                                                y2[:, c0:c0 + D],
                                                recip[:, idx:idx + 1])
                yns.append((i, yn))

            def emit_yT(hp, trs):
                nc.vector.tensor_scalar_add(
                    yT_sb[hp][:],
                    trs[:],
                    bqkv_sb[:, 2 * KC + hp:2 * KC + hp + 1],
                )

            def new_segs():
                return {h: {} for h in range(H)}

            # ---- cold start: qk(0) first, then S(0) groups (j0-first so
            # AV(0,0) unblocks early) with 1:1 v backfill ----
            segs = {0: new_segs()}
            for which, tchunk in (("q", 0), ("q", 1), ("k", 0), ("k", 1)):
                emit_qk_half(0, which, tchunk, on_act=True)
            for g in range(4):
                emit_S_group(0, segs[0], GROUPS[4 - g])
                emit_v(g)
            emit_S_j4_pair(0, segs[0])
            emit_v(4)
            emit_v(5)

            # ---- pipelined cycles ----
            # per cycle: 8 AV i-steps; qk(hp+1) halves at steps 0,1,3; S(hp+1)
            # groups j0-first at steps 2,4,5,6,7 (j0 consumed first next cycle).
            qk_sched = {0: [("q", 0), ("k", 0)], 1: [("q", 1)], 2: [("k", 1)]}
            def emit_transpose_slice(trs, yns, sl):
                for i, yn in yns[sl]:
                    nc.tensor.transpose(trs[:, 128 * i:128 * (i + 1)],
                                        yn[:], ident_sb[:])

            def emit_proj_mms(ps, cc, a, b_, kcs, stop_kc):
                for kc in kcs:
                    nc.tensor.matmul(
                        ps[:, a:b_],
                        wproj_sb[kc][:, cc * 128:(cc + 1) * 128],
                        yT_sb[kc][:, a:b_],
                        start=(kc == 0), stop=(kc == stop_kc),
                    )

            # staged projection partials: per cc, kc0-3 accumulated in a
            # transient big tile, then ACT-copied (+bias, fp32) to SBUF
            proj_part = {}
            proj_tmp = {}

            def unit_proj_half(cc, half):
                def f():
                    if half == 0:
                        proj_tmp[cc] = ps_big.tile([128, 1024], fp32,
                                                   tag="big", name="o_part_ps")
                    emit_proj_mms(proj_tmp[cc], cc, 512 * half,
                                  512 * (half + 1), range(4), None)
                    if half == 1:
                        part = osb.tile([128, T], fp32, tag="o_part",
                                        name="o_part", bufs=6)
                        nc.scalar.activation(part[:], proj_tmp.pop(cc)[:],
                                             Identity,
                                             bias=bproj_sb[:, cc:cc + 1])
                        proj_part[cc] = part
                return f

            proj_units = [unit_proj_half(cc, half)
                          for cc in range(KC) for half in (0, 1)]

            prev_yns = None
            for hp in range(NPAIR):
                nxt = hp + 1 < NPAIR
                last = not nxt
                if nxt:
                    segs[hp + 1] = new_segs()
                y2 = None
                yns = []
                trs = ps_tr.tile([128, 1024], bf16, tag="tr", name="tr")                     if prev_yns is not None else None
                for i in range(TT):
                    if i % 2 == 0:
                        if i == 0 and hp > 0:
                            # borrow an idle big-pool bank so AV(i=0,1) need
                            # not wait for the previous pair's y2 normalize
                            y2 = ps_big.tile([128, 1024], fp32, tag="big",
                                             name="y2big")[:, 0:512]
                        else:
                            y2 = ps_sm.tile([128, 512], fp32, tag="sm",
                                            name="y2")
                    emit_AV_half(hp, segs[hp], yns, i, y2, i % 2)
                    if hp == 0 and i in (3, 5):
                        emit_v(6 if i == 3 else 7)
                    if prev_yns is not None:
                        emit_transpose_slice(trs, prev_yns,
                                             slice(i, i + 1))
                    if nxt:
                        for args in qk_sched.get(i, []):
                            emit_qk_half(hp + 1, *args)
                        gidx = {2: 4, 3: 3, 4: 2, 5: 1}.get(i)
                        if gidx is not None:
                            emit_S_group(hp + 1, segs[hp + 1], GROUPS[gidx])
                        elif i == 6:
                            emit_S_j4_pair(hp + 1, segs[hp + 1])
                    elif last and proj_units:
                        # last pair: drain projection-partial units (kc 0-3;
                        # yT[0..3] are final) into the now-idle big pool /
                        # SBUF staging tiles
                        for u in (proj_units.pop(0) for _ in
                                  range(min(2, len(proj_units)))):
                            u()
                if prev_yns is not None:
                    emit_yT(hp - 1, trs)
                prev_yns = yns
                segs.pop(hp)
                if last:
                    while proj_units:
                        proj_units.pop(0)()
            trs = ps_tr.tile([128, 1024], bf16, tag="tr", name="tr")
            emit_transpose_slice(trs, prev_yns, slice(0, 8))
            emit_yT(NPAIR - 1, trs)

            # ---- projection endgame: kc4-5 accumulation + DVE merge with
            # the staged bias-carrying kc0-3 partials ----
            for cc in range(KC):
                ps = ps_big.tile([128, 1024], fp32, tag="big", name="o_ps")
                for a, b_ in ((0, 512), (512, 1024)):
                    for kc in (4, 5):
                        nc.tensor.matmul(
                            ps[:, a:b_],
                            wproj_sb[kc][:, cc * 128:(cc + 1) * 128],
                            yT_sb[kc][:, a:b_],
                            start=(kc == 4), stop=(kc == 5),
                        )
                o = osb.tile([128, T], bf16, tag="o_sb", name="o_sb")
                nc.vector.tensor_add(o[:], ps[:], proj_part[cc][:])
                nc.sync.dma_start(out[cc * 128:(cc + 1) * 128, :], o[:])

    nc.compile()
    return nc


def _prep_inputs(x, w_qkv, b_qkv, w_proj, b_proj):
    wqkv_bf = np.ascontiguousarray(w_qkv.astype(_BF16))
    wproj_bf = np.ascontiguousarray(w_proj.astype(_BF16))
    bqkv_pc = np.ascontiguousarray(b_qkv.astype(np.float32).reshape(C3 // 128, 128).T)
    bproj_pc = np.ascontiguousarray(b_proj.astype(np.float32).reshape(C // 128, 128).T)
    in_maps = []
    for b in range(B):
        xTb = np.ascontiguousarray(x[b].astype(_BF16).T)
        in_maps.append({
            "xT": xTb,
            "wqkv": wqkv_bf,
            "wproj": wproj_bf,
            "bqkv": bqkv_pc,
            "bproj": bproj_pc,
        })
    return in_maps


def _run(inputs, trace=False):
    from concourse.bass_utils import run_bass_kernel_spmd

    if "nc" not in _compiled:
        _compiled["nc"] = _build()
    nc = _compiled["nc"]
    in_maps = _prep_inputs(inputs["x"], inputs["w_qkv"], inputs["b_qkv"],
                           inputs["w_proj"], inputs["b_proj"])
    res = run_bass_kernel_spmd(nc, in_maps, list(range(B)), trace=trace)
    outs = np.stack([np.asarray(res.results[b]["out"]).T for b in range(B)])
    return outs.astype(np.float32), res


def kernel(x, w_qkv, b_qkv, w_proj, b_proj):
    outs, _ = _run(dict(x=x, w_qkv=w_qkv, b_qkv=b_qkv,
                        w_proj=w_proj, b_proj=b_proj))
    return outs



# revision 5
# speedup vs baseline: 1.1051x; 1.1051x over previous
"""Causal self-attention (B=8, T=1024, C=768, H=12, D=64) on 8 TRN2 NeuronCores.

Sharding: data-parallel over batch — core b handles batch element b. No
collectives. Host pre-transposes x to x^T[b] and pre-casts operands to fp16;
matmuls run fp16 with fp32 PSUM accumulation, except S = K Q^T which runs
fp8e4m3 in DoubleRow perf mode (0.5 cycles/row): both operands present their
single 64-channel contraction plane twice via a stride-0 broadcast dim, so
the DoubleRow two-plane sum computes exactly 2*S, folded into the exp scale
(1/16 instead of 1/8).

Per-core algorithm:
  v = x Wv in [t, c] layout (x^T stationary); v bias folded into the y^T
  stage (exact: softmax rows sum to 1). q^T,k^T = (Wqkv^T x^T + b) in
  [c3, t] layout (weights stationary), quantized to fp8e4m3 at the bias add.
  Per head h, key-block j (128 keys): S^T = K_j Q^T in PSUM [keys, q]
  (causal: only q >= 128j columns; blocks pack into [128,1024] PSUM tiles as
  {j0},{j1,j7},{j2,j6},{j3,j5},{j4} so one ACT exp covers each tile,
  scale=1/16), triangular mask-multiply on diagonal 128x128 blocks. Per
  q-tile i: y'[q,65] = sum_j P_j^T.T @ [V_j|1] accumulated in PSUM (two
  heads x two i-steps share one PSUM bank); col 64 is the softmax
  denominator. Normalize via per-partition reciprocal+scale, PE-transpose
  into a per-pair [128, 1024] fp16 PSUM strip (head parity in partition
  halves), one DVE pass per pair adds the v-bias and lands y^T in SBUF.
  out[t, c] = y^T.T @ Wproj + b_proj.

Emission is software-pipelined (static per-engine order => head-of-line
blocking): cycle hp interleaves AV(hp) i-steps with qk(hp+1) half-chunks
and S(hp+1) groups so PE fills ACT-paced exp stalls; v tiles fill the S(0)
cold start.

PSUM budget (8 banks): big [128,1024]fp32 x2 (4, shared v/S/o) +
qk [128,512]fp32 x2 (2, qk halves + y' i=0 borrow) +
y' [128,512]fp32 x1 (1, two heads x two i-steps packed) +
tr [128,1024]fp16 x1 (1).
"""

import numpy as np

B, T, C = 8, 1024, 768
H, D = 12, 64
C3 = 3 * C
KC = C // 128          # 6 contraction chunks over c_in
TT = T // 128          # 8 t-tiles of 128
NPAIR = H // 2

BIG_BUFS = 2
QK_BUFS = 2
SM_BUFS = 1
TR_BUFS = 1
PP_BUFS = 20           # 10 P segs live per pair (5 groups x 2 heads)

_F16 = np.float16

_compiled = {}


def _build():
    from concourse import bacc, mybir
    import concourse.tile as tile
    from concourse.masks import make_identity, make_upper_triangular

    fp32 = mybir.dt.float32
    f16 = mybir.dt.float16
    f8 = mybir.dt.float8e4
    DR = mybir.MatmulPerfMode.DoubleRow

    nc = bacc.Bacc("TRN2", target_bir_lowering=False, debug=False,
                   enable_asserts=True, num_devices=B)

    xT = nc.dram_tensor("xT", [C, T], f16, kind="ExternalInput")
    wqkv = nc.dram_tensor("wqkv", [C, C3], f16, kind="ExternalInput")
    wproj = nc.dram_tensor("wproj", [C, C], f16, kind="ExternalInput")
    # b_qkv rearranged host-side to [128, 18]: col j holds b_qkv[128j:128j+128]
    bqkv = nc.dram_tensor("bqkv", [128, C3 // 128], fp32, kind="ExternalInput")
    # b_proj rearranged host-side to [128, 6]: col j holds b_proj[128j:128j+128]
    bproj = nc.dram_tensor("bproj", [128, C // 128], fp32, kind="ExternalInput")
    # out is produced transposed [C, T] in f16; host transposes + casts back
    out = nc.dram_tensor("out", [C, T], f16, kind="ExternalOutput")

    Exp = mybir.ActivationFunctionType.Exp
    # S-block packing: groups of (j, base column) sharing one [128,1024]
    # PSUM tile => one exp per tile. Bases keep each block inside the tile.
    GROUPS = [((4, 0),), ((3, 0), (5, 640)), ((2, 0), (6, 768)),
              ((1, 0), (7, 896)), ((0, 0),)]

    def dr2(ap):
        # present a 2D AP as [p, 2, n] with a stride-0 plane dim: DoubleRow
        # then sums the same plane twice => computes 2x the matmul.
        p, n = ap.shape
        return ap.rearrange("p (o t) -> p o t", o=1).to_broadcast([p, 2, n])

    with tile.TileContext(nc) as tc:
        with (
            tc.tile_pool(name="const", bufs=1) as const,
            tc.tile_pool(name="pP", bufs=PP_BUFS) as pP,
            tc.tile_pool(name="small", bufs=6) as small,
            tc.tile_pool(name="osb", bufs=4) as osb,
            tc.tile_pool(name="ps_big", bufs=BIG_BUFS, space="PSUM") as ps_big,
            tc.tile_pool(name="ps_qk", bufs=QK_BUFS, space="PSUM") as ps_qk,
            tc.tile_pool(name="ps_sm", bufs=SM_BUFS, space="PSUM") as ps_sm,
            tc.tile_pool(name="ps_tr", bufs=TR_BUFS, space="PSUM") as ps_tr,
        ):
            # ---- persistent SBUF loads ----
            # All input streaming via the two HWDGE queues (SP + ACT), in
            # need-order: hp0+1 q/k sliver cols + x^T first (qk(0)), then
            # v cols, hp2-5 q/k cols, wproj.  DMA_ENGINES is a serial
            # resource: issue order IS the arrival order.
            bqkv_sb = const.tile([128, C3 // 128], fp32, tag="bqkv", name="bqkv")
            wq_big = const.tile([128, KC, C3], f16, tag="wqkv", name="wqkv")
            wqkv_sb = [wq_big[:, kc] for kc in range(KC)]
            wqkv_src = wqkv.rearrange("(k p) c -> p k c", k=KC)
            xT_sb = [const.tile([128, T], f16, tag=f"xT{kc}", name=f"xT{kc}")
                     for kc in range(KC)]
            nc.sync.dma_start(wq_big[:, :, 0:256], wqkv_src[:, :, 0:256])
            nc.scalar.dma_start(xT_sb[0][:], xT[0:128, :])
            nc.sync.dma_start(wq_big[:, :, C:C + 256], wqkv_src[:, :, C:C + 256])
            nc.scalar.dma_start(xT_sb[1][:], xT[128:256, :])
            nc.sync.dma_start(xT_sb[2][:], xT[256:384, :])
            nc.scalar.dma_start(xT_sb[3][:], xT[384:512, :])
            nc.sync.dma_start(xT_sb[4][:], xT[512:640, :])
            nc.scalar.dma_start(xT_sb[5][:], xT[640:768, :])
            nc.sync.dma_start(bqkv_sb[:], bqkv[:, :])
            nc.sync.dma_start(wq_big[:, :, 2 * C:], wqkv_src[:, :, 2 * C:])
            nc.sync.dma_start(wq_big[:, :, 256:C], wqkv_src[:, :, 256:C])
            nc.sync.dma_start(wq_big[:, :, C + 256:2 * C],
                              wqkv_src[:, :, C + 256:2 * C])
            wproj_big = const.tile([128, KC, C], f16, tag="wproj", name="wproj")
            wproj_sb = [wproj_big[:, kc] for kc in range(KC)]
            nc.sync.dma_start(
                wproj_big[:],
                wproj.rearrange("(k p) c -> p k c", k=KC),
            )
            bproj_sb = const.tile([128, C // 128], fp32, tag="bproj", name="bproj")
            nc.sync.dma_start(bproj_sb[:], bproj[:, :])
            ident_sb = const.tile([128, 128], f16, tag="ident", name="ident")
            make_identity(nc, ident_sb[:])
            # keep columns m >= l (query >= key) on the diagonal block
            trimask_sb = const.tile([128, 128], f16, tag="trimask", name="trimask")
            make_upper_triangular(nc, trimask_sb[:], val=1.0, diag=True)

            qkT_sb = [const.tile([128, T], f8, tag=f"qkT{c3}", name=f"qkT{c3}")
                      for c3 in range(2 * KC)]
            # v packed [t, 12 heads x (64 + ones col)]
            v_sb = [const.tile([128, H, D + 1], f16, tag=f"v{tt}", name=f"v{tt}")
                    for tt in range(TT)]
            yT_sb = [const.tile([128, T], f16, tag=f"yT{kc}", name=f"yT{kc}")
                     for kc in range(KC)]

            def emit_v(tt):
                ps = ps_big.tile([128, 1024], fp32, tag="big", name="v_ps")
                for kc in range(KC):
                    nc.tensor.matmul(
                        ps[:, 0:512],
                        xT_sb[kc][:, tt * 128:(tt + 1) * 128],
                        wqkv_sb[kc][:, 2 * C:2 * C + 512],
                        start=(kc == 0), stop=(kc == KC - 1),
                    )
                    nc.tensor.matmul(
                        ps[:, 512:768],
                        xT_sb[kc][:, tt * 128:(tt + 1) * 128],
                        wqkv_sb[kc][:, 2 * C + 512:3 * C],
                        start=(kc == 0), stop=(kc == KC - 1),
                    )
                vv = v_sb[tt]
                nc.vector.tensor_copy(
                    vv[:, :, 0:D],
                    ps[:, 0:768].rearrange("p (h d) -> p h d", d=D),
                )
                nc.vector.memset(vv[:, :, D:D + 1], 1.0)

            Identity = mybir.ActivationFunctionType.Identity

            def emit_qk_half(hp, which, tchunk, on_act=False):
                c3 = hp if which == "q" else KC + hp
                ps = ps_qk.tile([128, 512], fp32, tag="qk", name="qk_ps")
                sl = slice(tchunk * 512, (tchunk + 1) * 512)
                for kc in range(KC):
                    nc.tensor.matmul(
                        ps[:],
                        wqkv_sb[kc][:, c3 * 128:(c3 + 1) * 128],
                        xT_sb[kc][:, sl],
                        start=(kc == 0), stop=(kc == KC - 1),
                    )
                if on_act:
                    nc.scalar.activation(qkT_sb[c3][:, sl], ps[:],
                                         Identity, bias=bqkv_sb[:, c3:c3 + 1])
                else:
                    nc.vector.tensor_scalar_add(
                        qkT_sb[c3][:, sl], ps[:], bqkv_sb[:, c3:c3 + 1],
                    )

            def emit_S_group(hp, segs, grp):
                qT = qkT_sb[hp]
                kT = qkT_sb[KC + hp]
                for h in (2 * hp, 2 * hp + 1):
                    poff = 64 * (h % 2)
                    S = ps_big.tile([128, 1024], fp32, tag="big", name="S")
                    span = 0
                    for j, base in grp:
                        qs = 128 * j
                        w = T - qs
                        span = base + w
                        first = base + min(512 - base % 512, w) if base < 512 \
                            else base + w
                        for a, b_ in ((base, first), (first, base + w)):
                            if b_ <= a:
                                continue
                            nc.tensor.matmul(
                                S[:, a:b_],
                                dr2(kT[poff:poff + 64, qs:qs + 128]),
                                dr2(qT[poff:poff + 64,
                                       qs + (a - base):qs + (b_ - base)]),
                                start=True, stop=True, perf_mode=DR,
                            )
                    P = pP.tile([128, 1024], f16, tag="P", name="P")
                    nc.scalar.activation(P[:, 0:span], S[:, 0:span], Exp,
                                         scale=0.0625)
                    for j, base in grp:
                        nc.gpsimd.tensor_mul(P[:, base:base + 128],
                                             P[:, base:base + 128],
                                             trimask_sb[:])
                        segs[h][j] = (P, base)

            def emit_S_j4_pair(hp, segs):
                # both heads' j4 block (512 cols each) share one PSUM tile
                # and one exp: halves the ACT op overhead for this group
                qT = qkT_sb[hp]
                kT = qkT_sb[KC + hp]
                S = ps_big.tile([128, 1024], fp32, tag="big", name="S")
                for idx, h in enumerate((2 * hp, 2 * hp + 1)):
                    poff = 64 * (h % 2)
                    nc.tensor.matmul(
                        S[:, 512 * idx:512 * idx + 512],
                        dr2(kT[poff:poff + 64, 512:640]),
                        dr2(qT[poff:poff + 64, 512:1024]),
                        start=True, stop=True, perf_mode=DR,
                    )
                P = pP.tile([128, 1024], f16, tag="P", name="P")
                nc.scalar.activation(P[:], S[:], Exp, scale=0.0625)
                for idx, h in enumerate((2 * hp, 2 * hp + 1)):
                    base = 512 * idx
                    nc.gpsimd.tensor_mul(P[:, base:base + 128],
                                         P[:, base:base + 128],
                                         trimask_sb[:])
                    segs[h][4] = (P, base)

            def emit_AV_half(hp, segs, yns, i, y2, half):
                pair = (2 * hp, 2 * hp + 1)
                b0 = 256 * half
                for idx, h in enumerate(pair):
                    c0 = b0 + 128 * idx
                    for j in range(i + 1):
                        P, base = segs[h][j]
                        off = base + 128 * (i - j)
                        nc.tensor.matmul(
                            y2[:, c0:c0 + D + 1],
                            P[:, off:off + 128],
                            v_sb[j][:, h, :],
                            start=(j == 0), stop=(j == i),
                        )
                recip = small.tile([128, 2], fp32, tag="recip", name="recip")
                nc.vector.reciprocal(
                    recip[:],
                    y2[:].rearrange("p (g c) -> p g c", c=128)[:, 2 * half:2 * half + 2, D],
                )
                # both heads' normalized y packed [128, 128] -> one transpose
                yn = small.tile([128, 2 * D], f16, tag="yn", name="yn",
                                bufs=10)
                for idx, h in enumerate(pair):
                    c0 = b0 + 128 * idx
                    nc.vector.tensor_scalar_mul(yn[:, idx * D:(idx + 1) * D],
                                                y2[:, c0:c0 + D],
                                                recip[:, idx:idx + 1])
                yns.append((i, yn))

            def emit_yT(hp, trs):
                nc.vector.tensor_scalar_add(
                    yT_sb[hp][:],
                    trs[:],
                    bqkv_sb[:, 2 * KC + hp:2 * KC + hp + 1],
                )

            def new_segs():
                return {h: {} for h in range(H)}

            # ---- cold start: qk(0) first, then S(0) groups (j0-first so
            # AV(0,0) unblocks early) with 1:1 v backfill ----
            segs = {0: new_segs()}
            emit_qk_half(0, "q", 0, on_act=True)
            emit_qk_half(0, "q", 1, on_act=True)
            emit_qk_half(0, "k", 0, on_act=True)
            emit_S_group(0, segs[0], GROUPS[4])
            emit_qk_half(0, "k", 1, on_act=True)
            for n, (which, tchunk) in enumerate(
                    (("q", 0), ("q", 1), ("k", 0), ("k", 1))):
                emit_qk_half(1, which, tchunk)
                if n < 3:
                    emit_S_group(0, segs[0], GROUPS[3 - n])
                else:
                    emit_S_j4_pair(0, segs[0])
                emit_v(n)
            emit_v(4)
            emit_v(5)

            # ---- pipelined cycles ----
            # per cycle: 8 AV i-steps; qk(hp+1) halves at steps 0,1,3; S(hp+1)
            # groups j0-first at steps 2,4,5,6,7 (j0 consumed first next cycle).
            qk_order = [("q", 0), ("q", 1), ("k", 0), ("k", 1)]
            def emit_transpose_slice(trs, yns, sl):
                for i, yn in yns[sl]:
                    nc.tensor.transpose(trs[:, 128 * i:128 * (i + 1)],
                                        yn[:], ident_sb[:])

            def emit_proj_mms(ps, cc, a, b_, kcs, stop_kc):
                for kc in kcs:
                    nc.tensor.matmul(
                        ps[:, a:b_],
                        wproj_sb[kc][:, cc * 128:(cc + 1) * 128],
                        yT_sb[kc][:, a:b_],
                        start=(kc == 0), stop=(kc == stop_kc),
                    )

            # staged projection partials: per cc, kc0-3 accumulated in a
            # transient big tile, then ACT-copied (+bias, fp32) to SBUF
            proj_part = {}
            proj_tmp = {}

            def unit_proj_half(cc, half):
                def f():
                    if half == 0:
                        proj_tmp[cc] = ps_big.tile([128, 1024], fp32,
                                                   tag="big", name="o_part_ps")
                    emit_proj_mms(proj_tmp[cc], cc, 512 * half,
                                  512 * (half + 1), range(4), None)
                    if half == 1:
                        part = osb.tile([128, T], fp32, tag="o_part",
                                        name="o_part", bufs=6)
                        nc.scalar.activation(part[:], proj_tmp.pop(cc)[:],
                                             Identity,
                                             bias=bproj_sb[:, cc:cc + 1])
                        proj_part[cc] = part
                return f

            proj_units = [unit_proj_half(cc, half)
                          for cc in range(KC) for half in (0, 1)]

            prev_yns = None
            for hp in range(NPAIR):
                nxt = hp + 1 < NPAIR
                last = not nxt
                if nxt:
                    segs[hp + 1] = new_segs()
                y2 = None
                yns = []
                trs = ps_tr.tile([128, 1024], f16, tag="tr", name="tr") \
                    if prev_yns is not None else None
                for i in range(TT):
                    if i % 2 == 0:
                        if i == 0 and hp > 0:
                            # borrow a qk-pool bank so AV(i=0,1) need not
                            # wait for the previous pair's y2 normalize
                            y2 = ps_qk.tile([128, 512], fp32, tag="qk",
                                            name="y2qk")
                        else:
                            y2 = ps_sm.tile([128, 512], fp32, tag="sm",
                                            name="y2")
                    emit_AV_half(hp, segs[hp], yns, i, y2, i % 2)
                    if hp == 0 and i in (3, 5):
                        emit_v(6 if i == 3 else 7)
                    if prev_yns is not None:
                        emit_transpose_slice(trs, prev_yns,
                                             slice(i, i + 1))
                    if hp + 2 < NPAIR and i <= 3:
                        emit_qk_half(hp + 2, *qk_order[i])
                    if nxt:
                        gidx = {0: 4, 1: 3, 2: 2, 3: 1}.get(i)
                        if gidx is not None:
                            emit_S_group(hp + 1, segs[hp + 1], GROUPS[gidx])
                        elif i == 4:
                            emit_S_j4_pair(hp + 1, segs[hp + 1])
                    elif last and proj_units:
                        # last pair: drain projection-partial units (kc 0-3;
                        # yT[0..3] are final) into the now-idle big pool /
                        # SBUF staging tiles
                        for u in (proj_units.pop(0) for _ in
                                  range(min(2, len(proj_units)))):
                            u()
                if prev_yns is not None:
                    emit_yT(hp - 1, trs)
                prev_yns = yns
                segs.pop(hp)
                if last:
                    while proj_units:
                        proj_units.pop(0)()
            trs = ps_tr.tile([128, 1024], f16, tag="tr", name="tr")
            emit_transpose_slice(trs, prev_yns, slice(0, 8))
            emit_yT(NPAIR - 1, trs)

            # ---- projection endgame: kc4-5 accumulation + DVE merge with
            # the staged bias-carrying kc0-3 partials ----
            for cc in range(KC):
                ps = ps_big.tile([128, 1024], fp32, tag="big", name="o_ps")
                for a, b_ in ((0, 512), (512, 1024)):
                    for kc in (4, 5):
                        nc.tensor.matmul(
                            ps[:, a:b_],
                            wproj_sb[kc][:, cc * 128:(cc + 1) * 128],
                            yT_sb[kc][:, a:b_],
                            start=(kc == 4), stop=(kc == 5),
                        )
                o = osb.tile([128, T], f16, tag="o_sb", name="o_sb")
                nc.vector.tensor_add(o[:], ps[:], proj_part[cc][:])
                nc.sync.dma_start(out[cc * 128:(cc + 1) * 128, :], o[:])

    nc.compile()
    return nc


def _prep_inputs(x, w_qkv, b_qkv, w_proj, b_proj):
    wqkv_f = np.ascontiguousarray(w_qkv.astype(_F16))
    wproj_f = np.ascontiguousarray(w_proj.astype(_F16))
    bqkv_pc = np.ascontiguousarray(
        b_qkv.astype(np.float32).reshape(C3 // 128, 128).T)
    bproj_pc = np.ascontiguousarray(
        b_proj.astype(np.float32).reshape(C // 128, 128).T)
    in_maps = []
    for b in range(B):
        xTb = np.ascontiguousarray(x[b].astype(_F16).T)
        in_maps.append({
            "xT": xTb,
            "wqkv": wqkv_f,
            "wproj": wproj_f,
            "bqkv": bqkv_pc,
            "bproj": bproj_pc,
        })
    return in_maps


def _run(inputs, trace=False):
    from concourse.bass_utils import run_bass_kernel_spmd

    if "nc" not in _compiled:
        _compiled["nc"] = _build()
    nc = _compiled["nc"]
    in_maps = _prep_inputs(inputs["x"], inputs["w_qkv"], inputs["b_qkv"],
                           inputs["w_proj"], inputs["b_proj"])
    res = run_bass_kernel_spmd(nc, in_maps, list(range(B)), trace=trace)
    outs = np.stack([np.asarray(res.results[b]["out"]).T for b in range(B)])
    return outs.astype(np.float32), res


def kernel(x, w_qkv, b_qkv, w_proj, b_proj):
    outs, _ = _run(dict(x=x, w_qkv=w_qkv, b_qkv=b_qkv,
                        w_proj=w_proj, b_proj=b_proj))
    return outs


# revision 9
# speedup vs baseline: 1.1178x; 1.0114x over previous
"""Causal self-attention (B=8, T=1024, C=768, H=12, D=64) on 8 TRN2 NeuronCores.

Sharding: data-parallel over batch — core b handles batch element b. No
collectives. Host pre-transposes x to x^T[b] and pre-casts operands to fp16;
matmuls run fp16 with fp32 PSUM accumulation, except S = K Q^T which runs
fp8e4m3 in DoubleRow perf mode (0.5 cycles/row): both operands present their
single 64-channel contraction plane twice via a stride-0 broadcast dim, so
the DoubleRow two-plane sum computes exactly 2*S, folded into the exp scale
(1/16 instead of 1/8).

Per-core algorithm:
  v = x Wv in [t, c] layout (x^T stationary); v bias folded into the y^T
  stage (exact: softmax rows sum to 1). q^T,k^T = (Wqkv^T x^T + b) in
  [c3, t] layout (weights stationary), quantized to fp8e4m3 at the bias add.
  Per head h, key-block j (128 keys): S^T = K_j Q^T in PSUM [keys, q]
  (causal: only q >= 128j columns; blocks pack into [128,1024] PSUM tiles as
  {j0},{j1,j7},{j2,j6},{j3,j5},{j4} so one ACT exp covers each tile,
  scale=1/16), triangular mask-multiply on diagonal 128x128 blocks. Per
  q-tile i: y'[q,65] = sum_j P_j^T.T @ [V_j|1] accumulated in PSUM (two
  heads x two i-steps share one PSUM bank); col 64 is the softmax
  denominator. Normalize via per-partition reciprocal+scale, PE-transpose
  into a per-pair [128, 1024] fp16 PSUM strip (head parity in partition
  halves), one DVE pass per pair adds the v-bias and lands y^T in SBUF.
  out[t, c] = y^T.T @ Wproj + b_proj.

Emission is software-pipelined (static per-engine order => head-of-line
blocking): cycle hp interleaves AV(hp) i-steps with qk(hp+1) half-chunks
and S(hp+1) groups so PE fills ACT-paced exp stalls; v tiles fill the S(0)
cold start.

PSUM budget (8 banks): big [128,1024]fp32 x2 (4, shared v/S/o) +
qk [128,512]fp32 x2 (2, qk halves + y' i=0 borrow) +
y' [128,512]fp32 x1 (1, two heads x two i-steps packed) +
tr [128,1024]fp16 x1 (1).
"""

import ml_dtypes
import numpy as np

B, T, C = 8, 1024, 768
H, D = 12, 64
C3 = 3 * C
KC = C // 128          # 6 contraction chunks over c_in
TT = T // 128          # 8 t-tiles of 128
NPAIR = H // 2

BIG_BUFS = 2
QK_BUFS = 2
SM_BUFS = 1
TR_BUFS = 1
PP_BUFS = 20           # 10 P segs live per pair (5 groups x 2 heads)

_F16 = np.float16
_F8 = ml_dtypes.float8_e4m3

_compiled = {}


def _build():
    from concourse import bacc, mybir
    import concourse.tile as tile
    from concourse.masks import make_identity, make_upper_triangular

    fp32 = mybir.dt.float32
    f16 = mybir.dt.float16
    f8 = mybir.dt.float8e4
    DR = mybir.MatmulPerfMode.DoubleRow

    nc = bacc.Bacc("TRN2", target_bir_lowering=False, debug=False,
                   enable_asserts=True, num_devices=B)

    xTh = nc.dram_tensor("xTh", [C, T], f8, kind="ExternalInput")
    xTl = nc.dram_tensor("xTl", [C, T], f8, kind="ExternalInput")
    wqkvh = nc.dram_tensor("wqkvh", [C, C3], f8, kind="ExternalInput")
    wqkvl = nc.dram_tensor("wqkvl", [C, C3], f8, kind="ExternalInput")
    wproj = nc.dram_tensor("wproj", [C, C], f16, kind="ExternalInput")
    # b_qkv rearranged host-side to [128, 18]: col j holds b_qkv[128j:128j+128]
    bqkv = nc.dram_tensor("bqkv", [128, C3 // 128], fp32, kind="ExternalInput")
    # b_proj rearranged host-side to [128, 6]: col j holds b_proj[128j:128j+128]
    bproj = nc.dram_tensor("bproj", [128, C // 128], fp32, kind="ExternalInput")
    # out is produced transposed [C, T] in f16; host transposes + casts back
    out = nc.dram_tensor("out", [C, T], f16, kind="ExternalOutput")

    Exp = mybir.ActivationFunctionType.Exp
    # S-block packing: groups of (j, base column) sharing one [128,1024]
    # PSUM tile => one exp per tile. Bases keep each block inside the tile.
    GROUPS = [((4, 0),), ((3, 0), (5, 640)), ((2, 0), (6, 768)),
              ((1, 0), (7, 896)), ((0, 0),)]

    def dr2(ap):
        # present a 2D AP as [p, 2, n] with a stride-0 plane dim: DoubleRow
        # then sums the same plane twice => computes 2x the matmul.
        p, n = ap.shape
        return ap.rearrange("p (o t) -> p o t", o=1).to_broadcast([p, 2, n])

    with tile.TileContext(nc) as tc:
        with (
            tc.tile_pool(name="const", bufs=1) as const,
            tc.tile_pool(name="pP", bufs=PP_BUFS) as pP,
            tc.tile_pool(name="small", bufs=6) as small,
            tc.tile_pool(name="osb", bufs=4) as osb,
            tc.tile_pool(name="ps_big", bufs=BIG_BUFS, space="PSUM") as ps_big,
            tc.tile_pool(name="ps_qk", bufs=QK_BUFS, space="PSUM") as ps_qk,
            tc.tile_pool(name="ps_sm", bufs=SM_BUFS, space="PSUM") as ps_sm,
            tc.tile_pool(name="ps_tr", bufs=TR_BUFS, space="PSUM") as ps_tr,
        ):
            # ---- persistent SBUF loads ----
            # All input streaming via the two HWDGE queues (SP + ACT), in
            # need-order: hp0+1 q/k sliver cols + x^T first (qk(0)), then
            # v cols, hp2-5 q/k cols, wproj.  DMA_ENGINES is a serial
            # resource: issue order IS the arrival order.
            bqkv_sb = const.tile([128, C3 // 128], fp32, tag="bqkv", name="bqkv")
            w8h_sb = const.tile([128, KC, C3], f8, tag="w8h", name="w8h")
            w8l_sb = const.tile([128, KC, C3], f8, tag="w8l", name="w8l")
            wh_src = wqkvh.rearrange("(k p) c -> p k c", k=KC)
            wl_src = wqkvl.rearrange("(k p) c -> p k c", k=KC)
            x8h_sb = const.tile([128, KC, T], f8, tag="x8h", name="x8h")
            x8l_sb = const.tile([128, KC, T], f8, tag="x8l", name="x8l")
            xh_src = xTh.rearrange("(k p) t -> p k t", k=KC)
            xl_src = xTl.rearrange("(k p) t -> p k t", k=KC)
            # need-order: qk(0/1) sliver cols + x8h first, then x8l tchunk0 +
            # v weight cols (cold-start v tiles), then the rest.
            nc.sync.dma_start(w8h_sb[:, :, 0:256], wh_src[:, :, 0:256])
            nc.scalar.dma_start(x8h_sb[:, :, 0:512], xh_src[:, :, 0:512])
            nc.sync.dma_start(w8l_sb[:, :, 0:256], wl_src[:, :, 0:256])
            nc.scalar.dma_start(x8h_sb[:, :, 512:1024], xh_src[:, :, 512:1024])
            nc.sync.dma_start(w8h_sb[:, :, C:C + 256], wh_src[:, :, C:C + 256])
            nc.sync.dma_start(w8l_sb[:, :, C:C + 256], wl_src[:, :, C:C + 256])
            nc.sync.dma_start(bqkv_sb[:], bqkv[:, :])
            nc.scalar.dma_start(x8l_sb[:, :, 0:512], xl_src[:, :, 0:512])
            nc.scalar.dma_start(w8h_sb[:, :, 2 * C:], wh_src[:, :, 2 * C:])
            nc.scalar.dma_start(w8l_sb[:, :, 2 * C:], wl_src[:, :, 2 * C:])
            nc.sync.dma_start(w8h_sb[:, :, 256:C], wh_src[:, :, 256:C])
            nc.sync.dma_start(w8l_sb[:, :, 256:C], wl_src[:, :, 256:C])
            nc.sync.dma_start(w8h_sb[:, :, C + 256:2 * C],
                              wh_src[:, :, C + 256:2 * C])
            nc.sync.dma_start(w8l_sb[:, :, C + 256:2 * C],
                              wl_src[:, :, C + 256:2 * C])
            nc.scalar.dma_start(x8l_sb[:, :, 512:1024], xl_src[:, :, 512:1024])
            wproj_big = const.tile([128, KC, C], f16, tag="wproj", name="wproj")
            wproj_sb = [wproj_big[:, kc] for kc in range(KC)]
            nc.scalar.dma_start(
                wproj_big[:],
                wproj.rearrange("(k p) c -> p k c", k=KC),
            )
            bproj_sb = const.tile([128, C // 128], fp32, tag="bproj", name="bproj")
            nc.sync.dma_start(bproj_sb[:], bproj[:, :])
            ident_sb = const.tile([128, 128], f16, tag="ident", name="ident")
            make_identity(nc, ident_sb[:])
            # keep columns m >= l (query >= key) on the diagonal block
            trimask_sb = const.tile([128, 128], f16, tag="trimask", name="trimask")
            make_upper_triangular(nc, trimask_sb[:], val=1.0, diag=True)

            qkT_sb = [const.tile([128, T], f8, tag=f"qkT{c3}", name=f"qkT{c3}")
                      for c3 in range(2 * KC)]
            # v packed [t, 12 heads x (64 + ones col)]
            v_sb = [const.tile([128, H, D + 1], f16, tag=f"v{tt}", name=f"v{tt}")
                    for tt in range(TT)]
            yT_sb = [const.tile([128, T], f16, tag=f"yT{kc}", name=f"yT{kc}")
                     for kc in range(KC)]

            def emit_v(tt):
                ps = ps_big.tile([128, 1024], fp32, tag="big", name="v_ps")
                tsl = slice(tt * 128, (tt + 1) * 128)
                terms = [(x8h_sb, w8h_sb), (x8h_sb, w8l_sb), (x8l_sb, w8h_sb)]
                n = 0
                for xs, ws in terms:
                    for cp in range(KC // 2):
                        kk = slice(2 * cp, 2 * cp + 2)
                        nc.tensor.matmul(
                            ps[:, 0:512],
                            xs[:, kk, tsl],
                            ws[:, kk, 2 * C:2 * C + 512],
                            start=(n == 0), stop=(n == 8), perf_mode=DR,
                        )
                        nc.tensor.matmul(
                            ps[:, 512:768],
                            xs[:, kk, tsl],
                            ws[:, kk, 2 * C + 512:3 * C],
                            start=(n == 0), stop=(n == 8), perf_mode=DR,
                        )
                        n += 1
                vv = v_sb[tt]
                nc.vector.tensor_scalar_mul(
                    vv[:, :, 0:D],
                    ps[:, 0:768].rearrange("p (h d) -> p h d", d=D),
                    1.0 / 64.0,
                )
                nc.vector.memset(vv[:, :, D:D + 1], 1.0)

            Identity = mybir.ActivationFunctionType.Identity

            def emit_qk_half(hp, which, tchunk, on_act=False):
                c3 = hp if which == "q" else KC + hp
                ps = ps_qk.tile([128, 512], fp32, tag="qk", name="qk_ps")
                sl = slice(tchunk * 512, (tchunk + 1) * 512)
                n = 0
                for ws in (w8h_sb, w8l_sb):
                    for cp in range(KC // 2):
                        kk = slice(2 * cp, 2 * cp + 2)
                        nc.tensor.matmul(
                            ps[:],
                            ws[:, kk, c3 * 128:(c3 + 1) * 128],
                            x8h_sb[:, kk, sl],
                            start=(n == 0), stop=(n == 5), perf_mode=DR,
                        )
                        n += 1
                if on_act:
                    nc.scalar.activation(qkT_sb[c3][:, sl], ps[:],
                                         Identity, bias=bqkv_sb[:, c3:c3 + 1])
                else:
                    nc.vector.tensor_scalar_add(
                        qkT_sb[c3][:, sl], ps[:], bqkv_sb[:, c3:c3 + 1],
                    )

            def emit_S_group(hp, segs, grp):
                qT = qkT_sb[hp]
                kT = qkT_sb[KC + hp]
                for h in (2 * hp, 2 * hp + 1):
                    poff = 64 * (h % 2)
                    S = ps_big.tile([128, 1024], fp32, tag="big", name="S")
                    span = 0
                    for j, base in grp:
                        qs = 128 * j
                        w = T - qs
                        span = base + w
                        first = base + min(512 - base % 512, w) if base < 512 \
                            else base + w
                        for a, b_ in ((base, first), (first, base + w)):
                            if b_ <= a:
                                continue
                            nc.tensor.matmul(
                                S[:, a:b_],
                                dr2(kT[poff:poff + 64, qs:qs + 128]),
                                dr2(qT[poff:poff + 64,
                                       qs + (a - base):qs + (b_ - base)]),
                                start=True, stop=True, perf_mode=DR,
                            )
                    P = pP.tile([128, 1024], f16, tag="P", name="P")
                    nc.scalar.activation(P[:, 0:span], S[:, 0:span], Exp,
                                         scale=2.0 ** -16)
                    for j, base in grp:
                        nc.gpsimd.tensor_mul(P[:, base:base + 128],
                                             P[:, base:base + 128],
                                             trimask_sb[:])
                        segs[h][j] = (P, base)

            def emit_S_j4_pair(hp, segs):
                # both heads' j4 block (512 cols each) share one PSUM tile
                # and one exp: halves the ACT op overhead for this group
                qT = qkT_sb[hp]
                kT = qkT_sb[KC + hp]
                S = ps_big.tile([128, 1024], fp32, tag="big", name="S")
                for idx, h in enumerate((2 * hp, 2 * hp + 1)):
                    poff = 64 * (h % 2)
                    nc.tensor.matmul(
                        S[:, 512 * idx:512 * idx + 512],
                        dr2(kT[poff:poff + 64, 512:640]),
                        dr2(qT[poff:poff + 64, 512:1024]),
                        start=True, stop=True, perf_mode=DR,
                    )
                P = pP.tile([128, 1024], f16, tag="P", name="P")
                nc.scalar.activation(P[:], S[:], Exp, scale=2.0 ** -16)
                for idx, h in enumerate((2 * hp, 2 * hp + 1)):
                    base = 512 * idx
                    nc.gpsimd.tensor_mul(P[:, base:base + 128],
                                         P[:, base:base + 128],
                                         trimask_sb[:])
                    segs[h][4] = (P, base)

            def emit_AV_half(hp, segs, yns, i, y2, half):
                pair = (2 * hp, 2 * hp + 1)
                b0 = 256 * half
                for idx, h in enumerate(pair):
                    c0 = b0 + 128 * idx
                    for j in range(i + 1):
                        P, base = segs[h][j]
                        off = base + 128 * (i - j)
                        nc.tensor.matmul(
                            y2[:, c0:c0 + D + 1],
                            P[:, off:off + 128],
                            v_sb[j][:, h, :],
                            start=(j == 0), stop=(j == i),
                        )
                recip = small.tile([128, 2], fp32, tag="recip", name="recip")
                nc.vector.reciprocal(
                    recip[:],
                    y2[:].rearrange("p (g c) -> p g c", c=128)[:, 2 * half:2 * half + 2, D],
                )
                # both heads' normalized y packed [128, 128] -> one transpose
                yn = small.tile([128, 2 * D], f16, tag="yn", name="yn",
                                bufs=10)
                for idx, h in enumerate(pair):
                    c0 = b0 + 128 * idx
                    nc.vector.tensor_scalar_mul(yn[:, idx * D:(idx + 1) * D],
                                                y2[:, c0:c0 + D],
                                                recip[:, idx:idx + 1])
                yns.append((i, yn))

            def emit_yT(hp, trs):
                nc.vector.tensor_scalar_add(
                    yT_sb[hp][:],
                    trs[:],
                    bqkv_sb[:, 2 * KC + hp:2 * KC + hp + 1],
                )

            def new_segs():
                return {h: {} for h in range(H)}

            # ---- cold start: qk(0) first, then S(0) groups (j0-first so
            # AV(0,0) unblocks early) with 1:1 v backfill ----
            segs = {0: new_segs()}
            emit_qk_half(0, "q", 0, on_act=True)
            emit_qk_half(0, "q", 1, on_act=True)
            emit_qk_half(0, "k", 0, on_act=True)
            emit_S_group(0, segs[0], GROUPS[4])
            emit_qk_half(0, "k", 1, on_act=True)
            for n, (which, tchunk) in enumerate(
                    (("q", 0), ("q", 1), ("k", 0), ("k", 1))):
                emit_qk_half(1, which, tchunk)
                if n < 3:
                    emit_S_group(0, segs[0], GROUPS[3 - n])
                else:
                    emit_S_j4_pair(0, segs[0])
                emit_v(n)
            emit_v(4)
            emit_v(5)

            # ---- pipelined cycles ----
            # per cycle: 8 AV i-steps; qk(hp+1) halves at steps 0,1,3; S(hp+1)
            # groups j0-first at steps 2,4,5,6,7 (j0 consumed first next cycle).
            qk_order = [("q", 0), ("q", 1), ("k", 0), ("k", 1)]
            def emit_transpose_slice(trs, yns, sl):
                for i, yn in yns[sl]:
                    nc.tensor.transpose(trs[:, 128 * i:128 * (i + 1)],
                                        yn[:], ident_sb[:])

            def emit_proj_mms(ps, cc, a, b_, kcs, stop_kc):
                for kc in kcs:
                    nc.tensor.matmul(
                        ps[:, a:b_],
                        wproj_sb[kc][:, cc * 128:(cc + 1) * 128],
                        yT_sb[kc][:, a:b_],
                        start=(kc == 0), stop=(kc == stop_kc),
                    )

            # staged projection partials: per cc, kc0-3 accumulated in a
            # transient big tile, then ACT-copied (+bias, fp32) to SBUF
            proj_part = {}
            proj_tmp = {}

            def unit_proj_half(cc, half):
                def f():
                    if half == 0:
                        proj_tmp[cc] = ps_big.tile([128, 1024], fp32,
                                                   tag="big", name="o_part_ps")
                    emit_proj_mms(proj_tmp[cc], cc, 512 * half,
                                  512 * (half + 1), range(4), None)
                    if half == 1:
                        part = osb.tile([128, T], fp32, tag="o_part",
                                        name="o_part", bufs=6)
                        nc.scalar.activation(part[:], proj_tmp.pop(cc)[:],
                                             Identity,
                                             bias=bproj_sb[:, cc:cc + 1])
                        proj_part[cc] = part
                return f

            proj_units = [unit_proj_half(cc, half)
                          for cc in range(KC) for half in (0, 1)]

            prev_yns = None
            for hp in range(NPAIR):
                nxt = hp + 1 < NPAIR
                last = not nxt
                if nxt:
                    segs[hp + 1] = new_segs()
                y2 = None
                yns = []
                trs = ps_tr.tile([128, 1024], f16, tag="tr", name="tr") \
                    if prev_yns is not None else None
                for i in range(TT):
                    if i % 2 == 0:
                        if i == 0 and hp > 0:
                            # borrow a qk-pool bank so AV(i=0,1) need not
                            # wait for the previous pair's y2 normalize
                            y2 = ps_qk.tile([128, 512], fp32, tag="qk",
                                            name="y2qk")
                        else:
                            y2 = ps_sm.tile([128, 512], fp32, tag="sm",
                                            name="y2")
                    emit_AV_half(hp, segs[hp], yns, i, y2, i % 2)
                    if hp == 0 and i in (3, 5):
                        emit_v(6 if i == 3 else 7)
                    if prev_yns is not None:
                        emit_transpose_slice(trs, prev_yns,
                                             slice(i, i + 1))
                    if hp + 2 < NPAIR and i <= 3:
                        emit_qk_half(hp + 2, *qk_order[i])
                    if nxt:
                        gidx = {0: 4, 1: 3, 2: 2, 3: 1}.get(i)
                        if gidx is not None:
                            emit_S_group(hp + 1, segs[hp + 1], GROUPS[gidx])
                        elif i == 4:
                            emit_S_j4_pair(hp + 1, segs[hp + 1])
                    elif last and proj_units:
                        # last pair: drain projection-partial units (kc 0-3;
                        # yT[0..3] are final) into the now-idle big pool /
                        # SBUF staging tiles
                        for u in (proj_units.pop(0) for _ in
                                  range(min(2, len(proj_units)))):
                            u()
                if prev_yns is not None:
                    emit_yT(hp - 1, trs)
                prev_yns = yns
                segs.pop(hp)
                if last:
                    while proj_units:
                        proj_units.pop(0)()
            trs = ps_tr.tile([128, 1024], f16, tag="tr", name="tr")
            emit_transpose_slice(trs, prev_yns, slice(0, 8))
            emit_yT(NPAIR - 1, trs)

            # ---- projection endgame: kc4-5 accumulation + DVE merge with
            # the staged bias-carrying kc0-3 partials ----
            for cc in range(KC):
                ps = ps_big.tile([128, 1024], fp32, tag="big", name="o_ps")
                for a, b_ in ((0, 512), (512, 1024)):
                    for kc in (4, 5):
                        nc.tensor.matmul(
                            ps[:, a:b_],
                            wproj_sb[kc][:, cc * 128:(cc + 1) * 128],
                            yT_sb[kc][:, a:b_],
                            start=(kc == 4), stop=(kc == 5),
                        )
                o = osb.tile([128, T], f16, tag="o_sb", name="o_sb")
                nc.vector.tensor_add(o[:], ps[:], proj_part[cc][:])
                nc.sync.dma_start(out[cc * 128:(cc + 1) * 128, :], o[:])

    nc.compile()
    return nc


def _split_f8(a):
    hi = a.astype(_F8)
    lo = (a - hi.astype(np.float32)).astype(_F8)
    return np.ascontiguousarray(hi), np.ascontiguousarray(lo)


def _prep_inputs(x, w_qkv, b_qkv, w_proj, b_proj):
    # w scaled by 64 so fp8e4m3 quantization of the ~0.02-scale weights (and
    # their residuals) stays in the normal range; q/k biases scale to match
    # (exp scale folds the 64^2 back out); v descales at the on-chip copy.
    w64 = (w_qkv.astype(np.float32)) * 64.0
    wqkv_h, wqkv_l = _split_f8(w64)
    wproj_f = np.ascontiguousarray(w_proj.astype(_F16))
    b_sc = b_qkv.astype(np.float32).copy()
    b_sc[:2 * C] *= 64.0
    bqkv_pc = np.ascontiguousarray(b_sc.reshape(C3 // 128, 128).T)
    bproj_pc = np.ascontiguousarray(
        b_proj.astype(np.float32).reshape(C // 128, 128).T)
    in_maps = []
    for b in range(B):
        xTb = np.ascontiguousarray(x[b].astype(np.float32).T)
        xh, xl = _split_f8(xTb)
        in_maps.append({
            "xTh": xh,
            "xTl": xl,
            "wqkvh": wqkv_h,
            "wqkvl": wqkv_l,
            "wproj": wproj_f,
            "bqkv": bqkv_pc,
            "bproj": bproj_pc,
        })
    return in_maps


def _run(inputs, trace=False):
    from concourse.bass_utils import run_bass_kernel_spmd

    if "nc" not in _compiled:
        _compiled["nc"] = _build()
    nc = _compiled["nc"]
    in_maps = _prep_inputs(inputs["x"], inputs["w_qkv"], inputs["b_qkv"],
                           inputs["w_proj"], inputs["b_proj"])
    res = run_bass_kernel_spmd(nc, in_maps, list(range(B)), trace=trace)
    outs = np.stack([np.asarray(res.results[b]["out"]).T for b in range(B)])
    return outs.astype(np.float32), res


def kernel(x, w_qkv, b_qkv, w_proj, b_proj):
    outs, _ = _run(dict(x=x, w_qkv=w_qkv, b_qkv=b_qkv,
                        w_proj=w_proj, b_proj=b_proj))
    return outs


# revision 18
# speedup vs baseline: 1.1880x; 1.0629x over previous
"""Causal self-attention (B=8, T=1024, C=768, H=12, D=64) on 8 TRN2 NeuronCores.

Sharding: data-parallel over batch — core b handles batch element b. No
collectives. Host pre-transposes x to x^T[b] and pre-casts operands to fp16;
matmuls run fp16 with fp32 PSUM accumulation, except S = K Q^T which runs
fp8e4m3 in DoubleRow perf mode (0.5 cycles/row): both operands present their
single 64-channel contraction plane twice via a stride-0 broadcast dim, so
the DoubleRow two-plane sum computes exactly 2*S, folded into the exp scale
(1/16 instead of 1/8).

Per-core algorithm:
  v = x Wv in [t, c] layout (x^T stationary); v bias folded into the y^T
  stage (exact: softmax rows sum to 1). q^T,k^T = (Wqkv^T x^T + b) in
  [c3, t] layout (weights stationary), quantized to fp8e4m3 at the bias add.
  Per head h, key-block j (128 keys): S^T = K_j Q^T in PSUM [keys, q]
  (causal: only q >= 128j columns; blocks pack into [128,1024] PSUM tiles as
  {j0},{j1,j7},{j2,j6},{j3,j5},{j4} so one ACT exp covers each tile,
  scale=1/16), triangular mask-multiply on diagonal 128x128 blocks. Per
  q-tile i: y'[q,65] = sum_j P_j^T.T @ [V_j|1] accumulated in PSUM (two
  heads x two i-steps share one PSUM bank); col 64 is the softmax
  denominator. Normalize via per-partition reciprocal+scale, PE-transpose
  into a per-pair [128, 1024] fp16 PSUM strip (head parity in partition
  halves), one DVE pass per pair adds the v-bias and lands y^T in SBUF.
  out[t, c] = y^T.T @ Wproj + b_proj.

Emission is software-pipelined (static per-engine order => head-of-line
blocking): cycle hp interleaves AV(hp) i-steps with qk(hp+1) half-chunks
and S(hp+1) groups so PE fills ACT-paced exp stalls; v tiles fill the S(0)
cold start.

PSUM budget (8 banks): big [128,1024]fp32 x2 (4, shared v/S/o) +
qk [128,512]fp32 x2 (2, qk halves + y' i=0 borrow) +
y' [128,512]fp32 x1 (1, two heads x two i-steps packed) +
tr [128,1024]fp16 x1 (1).
"""

import ml_dtypes
import numpy as np

B, T, C = 8, 1024, 768
H, D = 12, 64
C3 = 3 * C
KC = C // 128          # 6 contraction chunks over c_in
TT = T // 128          # 8 t-tiles of 128
NPAIR = H // 2

BIG_BUFS = 2
QK_BUFS = 1
SM_BUFS = 2
TR_BUFS = 1
PP_BUFS = 32           # 10 P segs/pair x 3 pairs in flight + slack

_F16 = np.float16
_F8 = ml_dtypes.float8_e4m3

_compiled = {}


def _build():
    from concourse import bacc, mybir
    import concourse.tile as tile
    from concourse.masks import make_identity, make_upper_triangular

    fp32 = mybir.dt.float32
    f16 = mybir.dt.float16
    f8 = mybir.dt.float8e4
    DR = mybir.MatmulPerfMode.DoubleRow

    nc = bacc.Bacc("TRN2", target_bir_lowering=False, debug=False,
                   enable_asserts=True, num_devices=B)

    xTh = nc.dram_tensor("xTh", [C, T], f8, kind="ExternalInput")
    xTl = nc.dram_tensor("xTl", [C, T], f8, kind="ExternalInput")
    wqkvh = nc.dram_tensor("wqkvh", [C, C3], f8, kind="ExternalInput")
    wqkvl = nc.dram_tensor("wqkvl", [C, C3], f8, kind="ExternalInput")
    wproj = nc.dram_tensor("wproj", [C, C], f16, kind="ExternalInput")
    # b_qkv rearranged host-side to [128, 18]: col j holds b_qkv[128j:128j+128]
    bqkv = nc.dram_tensor("bqkv", [128, C3 // 128], fp32, kind="ExternalInput")
    # b_proj rearranged host-side to [128, 6]: col j holds b_proj[128j:128j+128]
    bproj = nc.dram_tensor("bproj", [128, C // 128], fp32, kind="ExternalInput")
    # out is produced transposed [C, T] in f16; host transposes + casts back
    out = nc.dram_tensor("out", [C, T], f16, kind="ExternalOutput")

    Exp = mybir.ActivationFunctionType.Exp
    # S-block packing: groups of (j, base column) sharing one [128,1024]
    # PSUM tile => one exp per tile. Bases keep each block inside the tile.
    GROUPS = [((4, 0),), ((3, 0), (5, 640)), ((2, 0), (6, 768)),
              ((1, 0), (7, 896)), ((0, 0),)]

    def dr2(ap):
        # present a 2D AP as [p, 2, n] with a stride-0 plane dim: DoubleRow
        # then sums the same plane twice => computes 2x the matmul.
        p, n = ap.shape
        return ap.rearrange("p (o t) -> p o t", o=1).to_broadcast([p, 2, n])

    with tile.TileContext(nc) as tc:
        with (
            tc.tile_pool(name="const", bufs=1) as const,
            tc.tile_pool(name="pP", bufs=PP_BUFS) as pP,
            tc.tile_pool(name="small", bufs=6) as small,
            tc.tile_pool(name="osb", bufs=4) as osb,
            tc.tile_pool(name="ps_big", bufs=BIG_BUFS, space="PSUM") as ps_big,
            tc.tile_pool(name="ps_qk", bufs=QK_BUFS, space="PSUM") as ps_qk,
            tc.tile_pool(name="ps_sm", bufs=SM_BUFS, space="PSUM") as ps_sm,
            tc.tile_pool(name="ps_tr", bufs=TR_BUFS, space="PSUM") as ps_tr,
        ):
            # ---- persistent SBUF loads ----
            # All input streaming via the two HWDGE queues (SP + ACT), in
            # need-order: hp0+1 q/k sliver cols + x^T first (qk(0)), then
            # v cols, hp2-5 q/k cols, wproj.  DMA_ENGINES is a serial
            # resource: issue order IS the arrival order.
            bqkv_sb = const.tile([128, C3 // 128], fp32, tag="bqkv", name="bqkv")
            w8h_sb = const.tile([128, KC, C3], f8, tag="w8h", name="w8h")
            w8l_sb = const.tile([128, KC, C3], f8, tag="w8l", name="w8l")
            wh_src = wqkvh.rearrange("(k p) c -> p k c", k=KC)
            wl_src = wqkvl.rearrange("(k p) c -> p k c", k=KC)
            x8h_sb = const.tile([128, KC, T], f8, tag="x8h", name="x8h")
            x8l_sb = const.tile([128, KC, T], f8, tag="x8l", name="x8l")
            xh_src = xTh.rearrange("(k p) t -> p k t", k=KC)
            xl_src = xTl.rearrange("(k p) t -> p k t", k=KC)
            # ALL input DMAs on the SP queue in exact need-order (HWDGE
            # generation and the DMA engine track are serial resources, and
            # scalar-queue DMAs would eat ACT SEQ time): qk(0/1) sliver
            # cols + x8h, then v weights + x8l, then pair-2+ slivers, wproj.
            nc.sync.dma_start(x8h_sb[:, :, 0:512], xh_src[:, :, 0:512])
            nc.sync.dma_start(w8h_sb[:, :, 0:256], wh_src[:, :, 0:256])
            nc.sync.dma_start(w8l_sb[:, :, 0:256], wl_src[:, :, 0:256])
            nc.sync.dma_start(w8h_sb[:, :, C:C + 256], wh_src[:, :, C:C + 256])
            nc.sync.dma_start(w8l_sb[:, :, C:C + 256], wl_src[:, :, C:C + 256])
            nc.sync.dma_start(x8h_sb[:, :, 512:1024], xh_src[:, :, 512:1024])
            nc.sync.dma_start(bqkv_sb[:], bqkv[:, :])
            nc.sync.dma_start(w8h_sb[:, :, 2 * C:], wh_src[:, :, 2 * C:])
            nc.sync.dma_start(w8l_sb[:, :, 2 * C:], wl_src[:, :, 2 * C:])
            nc.sync.dma_start(x8l_sb[:, :, 0:512], xl_src[:, :, 0:512])
            nc.sync.dma_start(x8l_sb[:, :, 512:1024], xl_src[:, :, 512:1024])
            nc.sync.dma_start(w8h_sb[:, :, 256:C], wh_src[:, :, 256:C])
            nc.sync.dma_start(w8l_sb[:, :, 256:C], wl_src[:, :, 256:C])
            nc.sync.dma_start(w8h_sb[:, :, C + 256:2 * C],
                              wh_src[:, :, C + 256:2 * C])
            nc.sync.dma_start(w8l_sb[:, :, C + 256:2 * C],
                              wl_src[:, :, C + 256:2 * C])
            bproj_sb = const.tile([128, C // 128], fp32, tag="bproj", name="bproj")
            nc.sync.dma_start(bproj_sb[:], bproj[:, :])
            wproj_big = const.tile([128, KC, C], f16, tag="wproj", name="wproj")
            wproj_sb = [wproj_big[:, kc] for kc in range(KC)]
            nc.sync.dma_start(
                wproj_big[:],
                wproj.rearrange("(k p) c -> p k c", k=KC),
            )
            ident_sb = const.tile([128, 128], f16, tag="ident", name="ident")
            make_identity(nc, ident_sb[:])
            # keep columns m >= l (query >= key) on the diagonal block
            trimask_sb = const.tile([128, 128], f16, tag="trimask", name="trimask")
            make_upper_triangular(nc, trimask_sb[:], val=1.0, diag=True)

            qkT_sb = [const.tile([128, T], f8, tag=f"qkT{c3}", name=f"qkT{c3}")
                      for c3 in range(2 * KC)]
            # v packed [t, 12 heads x (64 + ones col)]
            v_sb = [const.tile([128, H, D + 1], f16, tag=f"v{tt}", name=f"v{tt}")
                    for tt in range(TT)]
            yT_sb = [const.tile([128, T], f16, tag=f"yT{kc}", name=f"yT{kc}")
                     for kc in range(KC)]

            def emit_v(tt):
                ps = ps_big.tile([128, 1024], fp32, tag="big", name="v_ps")
                tsl = slice(tt * 128, (tt + 1) * 128)
                terms = [(x8h_sb, w8h_sb), (x8h_sb, w8l_sb), (x8l_sb, w8h_sb)]
                # x8l term last so v matmuls start before x8l lands
                n = 0
                for xs, ws in terms:
                    for cp in range(KC // 2):
                        kk = slice(2 * cp, 2 * cp + 2)
                        nc.tensor.matmul(
                            ps[:, 0:512],
                            xs[:, kk, tsl],
                            ws[:, kk, 2 * C:2 * C + 512],
                            start=(n == 0), stop=(n == 8), perf_mode=DR,
                        )
                        nc.tensor.matmul(
                            ps[:, 512:768],
                            xs[:, kk, tsl],
                            ws[:, kk, 2 * C + 512:3 * C],
                            start=(n == 0), stop=(n == 8), perf_mode=DR,
                        )
                        n += 1
                vv = v_sb[tt]
                nc.vector.tensor_scalar_mul(
                    vv[:, :, 0:D],
                    ps[:, 0:768].rearrange("p (h d) -> p h d", d=D),
                    1.0 / 64.0,
                )
                nc.vector.memset(vv[:, :, D:D + 1], 1.0)

            Identity = mybir.ActivationFunctionType.Identity

            def emit_qk_half(hp, which, tchunk, on_act=False):
                c3 = hp if which == "q" else KC + hp
                ps = ps_qk.tile([128, 512], fp32, tag="qk", name="qk_ps")
                sl = slice(tchunk * 512, (tchunk + 1) * 512)
                n = 0
                for ws in (w8h_sb, w8l_sb):
                    for cp in range(KC // 2):
                        kk = slice(2 * cp, 2 * cp + 2)
                        nc.tensor.matmul(
                            ps[:],
                            ws[:, kk, c3 * 128:(c3 + 1) * 128],
                            x8h_sb[:, kk, sl],
                            start=(n == 0), stop=(n == 5), perf_mode=DR,
                        )
                        n += 1
                if on_act:
                    nc.scalar.activation(qkT_sb[c3][:, sl], ps[:],
                                         Identity, bias=bqkv_sb[:, c3:c3 + 1])
                else:
                    nc.vector.tensor_scalar_add(
                        qkT_sb[c3][:, sl], ps[:], bqkv_sb[:, c3:c3 + 1],
                    )

            def emit_S_group(hp, segs, grp):
                qT = qkT_sb[hp]
                kT = qkT_sb[KC + hp]
                for h in (2 * hp, 2 * hp + 1):
                    poff = 64 * (h % 2)
                    S = ps_big.tile([128, 1024], fp32, tag="big", name="S")
                    span = 0
                    for j, base in grp:
                        qs = 128 * j
                        w = T - qs
                        span = base + w
                        first = base + min(512 - base % 512, w) if base < 512 \
                            else base + w
                        for a, b_ in ((base, first), (first, base + w)):
                            if b_ <= a:
                                continue
                            nc.tensor.matmul(
                                S[:, a:b_],
                                dr2(kT[poff:poff + 64, qs:qs + 128]),
                                dr2(qT[poff:poff + 64,
                                       qs + (a - base):qs + (b_ - base)]),
                                start=True, stop=True, perf_mode=DR,
                            )
                    P = pP.tile([128, 1024], f16, tag="P", name="P")
                    nc.scalar.activation(P[:, 0:span], S[:, 0:span], Exp,
                                         scale=2.0 ** -16)
                    for j, base in grp:
                        nc.gpsimd.tensor_mul(P[:, base:base + 128],
                                             P[:, base:base + 128],
                                             trimask_sb[:])
                        segs[h][j] = (P, base)

            def emit_S_j4_pair(hp, segs):
                # both heads' j4 block (512 cols each) share one PSUM tile
                # and one exp: halves the ACT op overhead for this group
                qT = qkT_sb[hp]
                kT = qkT_sb[KC + hp]
                S = ps_big.tile([128, 1024], fp32, tag="big", name="S")
                for idx, h in enumerate((2 * hp, 2 * hp + 1)):
                    poff = 64 * (h % 2)
                    nc.tensor.matmul(
                        S[:, 512 * idx:512 * idx + 512],
                        dr2(kT[poff:poff + 64, 512:640]),
                        dr2(qT[poff:poff + 64, 512:1024]),
                        start=True, stop=True, perf_mode=DR,
                    )
                P = pP.tile([128, 1024], f16, tag="P", name="P")
                nc.scalar.activation(P[:], S[:], Exp, scale=2.0 ** -16)
                for idx, h in enumerate((2 * hp, 2 * hp + 1)):
                    base = 512 * idx
                    nc.gpsimd.tensor_mul(P[:, base:base + 128],
                                         P[:, base:base + 128],
                                         trimask_sb[:])
                    segs[h][4] = (P, base)

            def emit_AV_half(hp, segs, yns, i, y2, half):
                pair = (2 * hp, 2 * hp + 1)
                b0 = 256 * half
                for idx, h in enumerate(pair):
                    c0 = b0 + 128 * idx
                    for j in range(i + 1):
                        P, base = segs[h][j]
                        off = base + 128 * (i - j)
                        nc.tensor.matmul(
                            y2[:, c0:c0 + D + 1],
                            P[:, off:off + 128],
                            v_sb[j][:, h, :],
                            start=(j == 0), stop=(j == i),
                        )
                recip = small.tile([128, 2], fp32, tag="recip", name="recip")
                nc.vector.reciprocal(
                    recip[:],
                    y2[:].rearrange("p (g c) -> p g c", c=128)[:, 2 * half:2 * half + 2, D],
                )
                # both heads' normalized y packed [128, 128] -> one transpose
                yn = small.tile([128, 2 * D], f16, tag="yn", name="yn",
                                bufs=10)
                for idx, h in enumerate(pair):
                    c0 = b0 + 128 * idx
                    nc.vector.tensor_scalar_mul(yn[:, idx * D:(idx + 1) * D],
                                                y2[:, c0:c0 + D],
                                                recip[:, idx:idx + 1])
                yns.append((i, yn))

            def emit_yT(hp, trs):
                nc.vector.tensor_scalar_add(
                    yT_sb[hp][:],
                    trs[:],
                    bqkv_sb[:, 2 * KC + hp:2 * KC + hp + 1],
                )

            def new_segs():
                return {h: {} for h in range(H)}

            # ---- cold start: qk(0) first, then S(0) groups (j0-first so
            # AV(0,0) unblocks early) with 1:1 v backfill ----
            segs = {0: new_segs(), 1: new_segs()}
            emit_qk_half(0, "q", 0)
            emit_qk_half(0, "k", 0)
            emit_qk_half(0, "q", 1)
            emit_S_group(0, segs[0], GROUPS[4])
            emit_qk_half(0, "k", 1)
            for n, (which, tchunk) in enumerate(
                    (("q", 0), ("q", 1), ("k", 0), ("k", 1))):
                emit_qk_half(1, which, tchunk)
                if n < 3:
                    emit_S_group(0, segs[0], GROUPS[3 - n])
                else:
                    emit_S_j4_pair(0, segs[0])
            # v tiles before S(1): v's consumers (DVE copies) release big-
            # pool slots fast, while S tiles are released by ACT exps — v
            # first keeps the ring exp-decoupled and AV(0) unblocks early.
            # S(1)'s exps then keep ACT busy into pair 0 (depth-2 S/exp
            # pipeline: S(hp+2) is emitted during pair hp).
            emit_S_group(1, segs[1], GROUPS[4])
            emit_v(0)
            emit_S_group(1, segs[1], GROUPS[3])
            emit_v(1)
            emit_S_group(1, segs[1], GROUPS[2])
            emit_v(2)
            emit_S_group(1, segs[1], GROUPS[1])
            emit_v(3)
            emit_S_j4_pair(1, segs[1])
            emit_v(4)
            emit_v(5)

            # ---- pipelined cycles ----
            # per cycle: 8 AV i-steps; qk(hp+1) halves at steps 0,1,3; S(hp+1)
            # groups j0-first at steps 2,4,5,6,7 (j0 consumed first next cycle).
            qk_order = [("q", 0), ("q", 1), ("k", 0), ("k", 1)]
            def emit_transpose_slice(trs, yns, sl):
                for i, yn in yns[sl]:
                    nc.tensor.transpose(trs[:, 128 * i:128 * (i + 1)],
                                        yn[:], ident_sb[:])

            def emit_proj_mms(ps, cc, a, b_, kcs, stop_kc):
                for kc in kcs:
                    nc.tensor.matmul(
                        ps[:, a:b_],
                        wproj_sb[kc][:, cc * 128:(cc + 1) * 128],
                        yT_sb[kc][:, a:b_],
                        start=(kc == 0), stop=(kc == stop_kc),
                    )

            # staged projection partials: per cc, kc0-3 accumulated in a
            # transient big tile, then ACT-copied (+bias, fp32) to SBUF
            proj_part = {}
            proj_tmp = {}

            def unit_proj_half(cc, half):
                def f():
                    if half == 0:
                        proj_tmp[cc] = ps_big.tile([128, 1024], fp32,
                                                   tag="big", name="o_part_ps")
                    emit_proj_mms(proj_tmp[cc], cc, 512 * half,
                                  512 * (half + 1), range(3), None)
                    if half == 1:
                        part = osb.tile([128, T], fp32, tag="o_part",
                                        name="o_part", bufs=6)
                        nc.scalar.activation(part[:], proj_tmp.pop(cc)[:],
                                             Identity,
                                             bias=bproj_sb[:, cc:cc + 1])
                        proj_part[cc] = part
                return f

            proj_units = [unit_proj_half(cc, half)
                          for cc in range(KC) for half in (0, 1)]

            prev_yns = None
            for hp in range(NPAIR):
                nxt = hp + 1 < NPAIR
                last = not nxt
                if hp + 2 < NPAIR:
                    segs[hp + 2] = new_segs()
                y2 = None
                yns = []
                trs = ps_tr.tile([128, 1024], f16, tag="tr", name="tr") \
                    if prev_yns is not None else None
                for i in range(TT):
                    if i % 2 == 0:
                        y2 = ps_sm.tile([128, 512], fp32, tag="sm",
                                        name="y2")
                    emit_AV_half(hp, segs[hp], yns, i, y2, i % 2)
                    if hp == 0 and i in (3, 5):
                        emit_v(6 if i == 3 else 7)
                    if prev_yns is not None:
                        emit_transpose_slice(trs, prev_yns,
                                             slice(i, i + 1))
                    if hp + 2 < NPAIR and i <= 3:
                        emit_qk_half(hp + 2, *qk_order[i])
                    if hp + 2 < NPAIR:
                        gidx = {3: 4, 4: 3, 5: 2, 6: 1}.get(i)
                        if gidx is not None:
                            emit_S_group(hp + 2, segs[hp + 2], GROUPS[gidx])
                        elif i == 7:
                            emit_S_j4_pair(hp + 2, segs[hp + 2])
                    elif hp == NPAIR - 2 and proj_units:
                        # pair 4 is qk/S-free: drain projection partials
                        # (kc 0-2; yT[0..2] final after pair 3) into the
                        # idle big pool
                        for u in (proj_units.pop(0) for _ in
                                  range(min(2, len(proj_units)))):
                            u()
                if prev_yns is not None:
                    emit_yT(hp - 1, trs)
                prev_yns = yns
                segs.pop(hp)
                if last:
                    while proj_units:
                        proj_units.pop(0)()
            trs = ps_tr.tile([128, 1024], f16, tag="tr", name="tr")
            emit_transpose_slice(trs, prev_yns, slice(0, 8))
            emit_yT(NPAIR - 1, trs)

            # ---- projection endgame: kc3-5 accumulation + DVE merge with
            # the staged bias-carrying kc0-2 partials ----
            for cc in range(KC):
                ps = ps_big.tile([128, 1024], fp32, tag="big", name="o_ps")
                for a, b_ in ((0, 512), (512, 1024)):
                    for kc in (3, 4, 5):
                        nc.tensor.matmul(
                            ps[:, a:b_],
                            wproj_sb[kc][:, cc * 128:(cc + 1) * 128],
                            yT_sb[kc][:, a:b_],
                            start=(kc == 3), stop=(kc == 5),
                        )
                o = osb.tile([128, T], f16, tag="o_sb", name="o_sb")
                nc.vector.tensor_add(o[:], ps[:], proj_part[cc][:])
                nc.sync.dma_start(out[cc * 128:(cc + 1) * 128, :], o[:])

    nc.compile()
    return nc


def _split_f8(a):
    hi = a.astype(_F8)
    lo = (a - hi.astype(np.float32)).astype(_F8)
    return np.ascontiguousarray(hi), np.ascontiguousarray(lo)


def _prep_inputs(x, w_qkv, b_qkv, w_proj, b_proj):
    # w scaled by 64 so fp8e4m3 quantization of the ~0.02-scale weights (and
    # their residuals) stays in the normal range; q/k biases scale to match
    # (exp scale folds the 64^2 back out); v descales at the on-chip copy.
    w64 = (w_qkv.astype(np.float32)) * 64.0
    wqkv_h, wqkv_l = _split_f8(w64)
    wproj_f = np.ascontiguousarray(w_proj.astype(_F16))
    b_sc = b_qkv.astype(np.float32).copy()
    b_sc[:2 * C] *= 64.0
    bqkv_pc = np.ascontiguousarray(b_sc.reshape(C3 // 128, 128).T)
    bproj_pc = np.ascontiguousarray(
        b_proj.astype(np.float32).reshape(C // 128, 128).T)
    in_maps = []
    for b in range(B):
        xTb = np.ascontiguousarray(x[b].astype(np.float32).T)
        xh, xl = _split_f8(xTb)
        in_maps.append({
            "xTh": xh,
            "xTl": xl,
            "wqkvh": wqkv_h,
            "wqkvl": wqkv_l,
            "wproj": wproj_f,
            "bqkv": bqkv_pc,
            "bproj": bproj_pc,
        })
    return in_maps


def _run(inputs, trace=False):
    from concourse.bass_utils import run_bass_kernel_spmd

    if "nc" not in _compiled:
        _compiled["nc"] = _build()
    nc = _compiled["nc"]
    in_maps = _prep_inputs(inputs["x"], inputs["w_qkv"], inputs["b_qkv"],
                           inputs["w_proj"], inputs["b_proj"])
    res = run_bass_kernel_spmd(nc, in_maps, list(range(B)), trace=trace)
    outs = np.stack([np.asarray(res.results[b]["out"]).T for b in range(B)])
    return outs.astype(np.float32), res


def kernel(x, w_qkv, b_qkv, w_proj, b_proj):
    outs, _ = _run(dict(x=x, w_qkv=w_qkv, b_qkv=b_qkv,
                        w_proj=w_proj, b_proj=b_proj))
    return outs


# revision 30
# speedup vs baseline: 1.2047x; 1.0140x over previous
"""Causal self-attention (B=8, T=1024, C=768, H=12, D=64) on 8 TRN2 NeuronCores.

Sharding: data-parallel over batch — core b handles batch element b. No
collectives. Host pre-transposes x to x^T[b] and pre-casts operands to fp16;
matmuls run fp16 with fp32 PSUM accumulation, except S = K Q^T which runs
fp8e4m3 in DoubleRow perf mode (0.5 cycles/row): both operands present their
single 64-channel contraction plane twice via a stride-0 broadcast dim, so
the DoubleRow two-plane sum computes exactly 2*S, folded into the exp scale
(1/16 instead of 1/8).

Per-core algorithm:
  v = x Wv in [t, c] layout (x^T stationary); v bias folded into the y^T
  stage (exact: softmax rows sum to 1). q^T,k^T = (Wqkv^T x^T + b) in
  [c3, t] layout (weights stationary), quantized to fp8e4m3 at the bias add.
  Per head h, key-block j (128 keys): S^T = K_j Q^T in PSUM [keys, q]
  (causal: only q >= 128j columns; blocks pack into [128,1024] PSUM tiles as
  {j0},{j1,j7},{j2,j6},{j3,j5},{j4} so one ACT exp covers each tile,
  scale=1/16), triangular mask-multiply on diagonal 128x128 blocks. Per
  q-tile i: y'[q,65] = sum_j P_j^T.T @ [V_j|1] accumulated in PSUM (two
  heads x two i-steps share one PSUM bank); col 64 is the softmax
  denominator. Normalize via per-partition reciprocal+scale, PE-transpose
  into a per-pair [128, 1024] fp16 PSUM strip (head parity in partition
  halves), one DVE pass per pair adds the v-bias and lands y^T in SBUF.
  out[t, c] = y^T.T @ Wproj + b_proj.

Emission is software-pipelined (static per-engine order => head-of-line
blocking): cycle hp interleaves AV(hp) i-steps with qk(hp+1) half-chunks
and S(hp+1) groups so PE fills ACT-paced exp stalls; v tiles fill the S(0)
cold start.

PSUM budget (8 banks): big [128,1024]fp32 x2 (4, shared v/S/o) +
shared [128,512]fp32 x3 (3, qk halves + y' two-heads-x-two-i-steps) +
tr [128,1024]fp16 x1 (1).
"""

import ml_dtypes
import numpy as np

B, T, C = 8, 1024, 768
H, D = 12, 64
C3 = 3 * C
KC = C // 128          # 6 contraction chunks over c_in
TT = T // 128          # 8 t-tiles of 128
NPAIR = H // 2

BIG_BUFS = 2
QK_BUFS = 1
SM_BUFS = 2
TR_BUFS = 1
PP_BUFS = 32           # 10 P segs/pair x 3 pairs in flight + slack

_F16 = np.float16
_F8 = ml_dtypes.float8_e4m3

_compiled = {}


def _build():
    from concourse import bacc, mybir
    import concourse.tile as tile
    from concourse.masks import make_identity, make_upper_triangular

    fp32 = mybir.dt.float32
    f16 = mybir.dt.float16
    f8 = mybir.dt.float8e4
    DR = mybir.MatmulPerfMode.DoubleRow

    nc = bacc.Bacc("TRN2", target_bir_lowering=False, debug=False,
                   enable_asserts=True, num_devices=B)

    # x: hi/lo fp8 planes interleaved; w: hi/lo in load-order column blocks
    # [q01h q01l k01h k01l vh vl q25h q25l k25h k25l] so each cold DMA is one
    # contiguous 3D copy
    xT8 = nc.dram_tensor("xT8", [C, 2, T], f8, kind="ExternalInput")
    wqkv8 = nc.dram_tensor("wqkv8", [C, 2 * C3], f8, kind="ExternalInput")
    wproj = nc.dram_tensor("wproj", [C, C], f16, kind="ExternalInput")
    # b_qkv rearranged host-side to [128, 18]: col j holds b_qkv[128j:128j+128]
    bqkv = nc.dram_tensor("bqkv", [128, C3 // 128], fp32, kind="ExternalInput")
    # b_proj rearranged host-side to [128, 6]: col j holds b_proj[128j:128j+128]
    bproj = nc.dram_tensor("bproj", [128, C // 128], fp32, kind="ExternalInput")
    # out is produced transposed [C, T] in f16; host transposes + casts back
    out = nc.dram_tensor("out", [C, T], f16, kind="ExternalOutput")

    Exp = mybir.ActivationFunctionType.Exp
    # S-block packing: groups of (j, base column) sharing one [128,1024]
    # PSUM tile => one exp per tile. Bases keep each block inside the tile.
    GROUPS = [((4, 0),), ((3, 0), (5, 640)), ((2, 0), (6, 768)),
              ((1, 0), (7, 896)), ((0, 0),)]

    def dr2(ap):
        # present a 2D AP as [p, 2, n] with a stride-0 plane dim: DoubleRow
        # then sums the same plane twice => computes 2x the matmul.
        p, n = ap.shape
        return ap.rearrange("p (o t) -> p o t", o=1).to_broadcast([p, 2, n])

    with tile.TileContext(nc) as tc:
        with (
            tc.tile_pool(name="const", bufs=1) as const,
            tc.tile_pool(name="pP", bufs=PP_BUFS) as pP,
            tc.tile_pool(name="small", bufs=6) as small,
            tc.tile_pool(name="osb", bufs=4) as osb,
            tc.tile_pool(name="ps_big", bufs=BIG_BUFS, space="PSUM") as ps_big,
            tc.tile_pool(name="ps_qk", bufs=QK_BUFS, space="PSUM") as ps_qk,
            tc.tile_pool(name="ps_sm", bufs=SM_BUFS, space="PSUM") as ps_sm,
            tc.tile_pool(name="ps_tr", bufs=TR_BUFS, space="PSUM") as ps_tr,
        ):
            # ---- persistent SBUF loads ----
            # All input streaming via the two HWDGE queues (SP + ACT), in
            # need-order: hp0+1 q/k sliver cols + x^T first (qk(0)), then
            # v cols, hp2-5 q/k cols, wproj.  DMA_ENGINES is a serial
            # resource: issue order IS the arrival order.
            bqkv_sb = const.tile([128, C3 // 128], fp32, tag="bqkv", name="bqkv")
            w8_sb = const.tile([128, KC, 2 * C3], f8, tag="w8", name="w8")
            w_src = wqkv8.rearrange("(k p) c -> p k c", k=KC)

            def wcol(hl, c3):
                # load-order flat column of sliver c3's first column
                if c3 < 2:
                    return 256 * hl + c3 * 128
                if 6 <= c3 < 8:
                    return 512 + 256 * hl + (c3 - 6) * 128
                if 2 <= c3 < 6:
                    return 2560 + 512 * hl + (c3 - 2) * 128
                return 3584 + 512 * hl + (c3 - 8) * 128

            WV = [1024, 1792]  # v-section bases (hi, lo)
            x8_sb = const.tile([128, KC, 2, T], f8, tag="x8", name="x8")
            x_src = xT8.rearrange("(k p) two t -> p k two t", k=KC)
            # ALL input DMAs on the SP queue in exact need-order (HWDGE
            # generation and the DMA engine track are serial resources, and
            # scalar-queue DMAs would eat ACT SEQ time): S(0)-j0's inputs
            # first (x8h both halves + q01/k01 slivers), then v weights +
            # x8l, then pair-2+ slivers, wproj.
            nc.sync.dma_start(bqkv_sb[:], bqkv[:, :])
            nc.sync.dma_start(x8_sb[:, :, 0, 0:512], x_src[:, :, 0, 0:512])
            nc.sync.dma_start(w8_sb[:, :, 0:512], w_src[:, :, 0:512])
            nc.sync.dma_start(x8_sb[:, :, 0, 512:1024],
                              x_src[:, :, 0, 512:1024])
            nc.sync.dma_start(w8_sb[:, :, 512:1024], w_src[:, :, 512:1024])
            nc.sync.dma_start(w8_sb[:, :, 1024:2560], w_src[:, :, 1024:2560])
            nc.sync.dma_start(x8_sb[:, :, 1, 0:512], x_src[:, :, 1, 0:512])
            nc.sync.dma_start(x8_sb[:, :, 1, 512:1024],
                              x_src[:, :, 1, 512:1024])
            nc.sync.dma_start(w8_sb[:, :, 2560:3584], w_src[:, :, 2560:3584])
            nc.sync.dma_start(w8_sb[:, :, 3584:4608], w_src[:, :, 3584:4608])
            bproj_sb = const.tile([128, C // 128], fp32, tag="bproj", name="bproj")
            nc.sync.dma_start(bproj_sb[:], bproj[:, :])
            wproj_big = const.tile([128, KC, C], f16, tag="wproj", name="wproj")
            wproj_sb = [wproj_big[:, kc] for kc in range(KC)]
            nc.sync.dma_start(
                wproj_big[:],
                wproj.rearrange("(k p) c -> p k c", k=KC),
            )
            ident_sb = const.tile([128, 128], f16, tag="ident", name="ident")
            make_identity(nc, ident_sb[:])
            # keep columns m >= l (query >= key) on the diagonal block
            trimask_sb = const.tile([128, 128], f16, tag="trimask", name="trimask")
            make_upper_triangular(nc, trimask_sb[:], val=1.0, diag=True)

            qkT_sb = [const.tile([128, T], f8, tag=f"qkT{c3}", name=f"qkT{c3}")
                      for c3 in range(2 * KC)]
            # v packed [t, 12 heads x (64 + ones col)]
            v_sb = [const.tile([128, H, D + 1], f16, tag=f"v{tt}", name=f"v{tt}")
                    for tt in range(TT)]
            yT_sb = [const.tile([128, T], f16, tag=f"yT{kc}", name=f"yT{kc}")
                     for kc in range(KC)]

            def emit_v(tt):
                # two sm-pool tiles (fast DVE-only consumers) keep v off the
                # exp-paced big ring
                psA = ps_sm.tile([128, 512], fp32, tag="sm", name="vA_ps")
                psB = ps_sm.tile([128, 512], fp32, tag="sm", name="vB_ps")
                tsl = slice(tt * 128, (tt + 1) * 128)
                terms = [(0, 0), (0, 1), (1, 0)]
                # x8l term last so v matmuls start before x8l lands
                n = 0
                for xs, ws in terms:
                    vb = WV[ws]
                    for cp in range(KC // 2):
                        kk = slice(2 * cp, 2 * cp + 2)
                        nc.tensor.matmul(
                            psA[:],
                            x8_sb[:, kk, xs, tsl],
                            w8_sb[:, kk, vb:vb + 512],
                            start=(n == 0), stop=(n == 8), perf_mode=DR,
                        )
                        nc.tensor.matmul(
                            psB[:, 0:256],
                            x8_sb[:, kk, xs, tsl],
                            w8_sb[:, kk, vb + 512:vb + 768],
                            start=(n == 0), stop=(n == 8), perf_mode=DR,
                        )
                        n += 1
                vv = v_sb[tt]
                nc.vector.tensor_scalar_mul(
                    vv[:, 0:8, 0:D],
                    psA[:].rearrange("p (h d) -> p h d", d=D),
                    1.0 / 64.0,
                )
                nc.vector.tensor_scalar_mul(
                    vv[:, 8:12, 0:D],
                    psB[:, 0:256].rearrange("p (h d) -> p h d", d=D),
                    1.0 / 64.0,
                )
                nc.vector.memset(vv[:, :, D:D + 1], 1.0)

            Identity = mybir.ActivationFunctionType.Identity

            def emit_qk_half(hp, which, tchunk, on_act=False, cold=False):
                c3 = hp if which == "q" else KC + hp
                # cold-start halves alternate the y2 pool (idle until pair
                # 0) with the dedicated bank: an effective ring-3 while DMA
                # streams
                if cold == "sm":
                    pool, tg = ps_sm, "sm"
                else:
                    pool, tg = ps_qk, "qk"
                ps = pool.tile([128, 512], fp32, tag=tg, name="qk_ps")
                sl = slice(tchunk * 512, (tchunk + 1) * 512)
                n = 0
                for ws in (0, 1):
                    wc = wcol(ws, c3)
                    for cp in range(KC // 2):
                        kk = slice(2 * cp, 2 * cp + 2)
                        nc.tensor.matmul(
                            ps[:],
                            w8_sb[:, kk, wc:wc + 128],
                            x8_sb[:, kk, 0, sl],
                            start=(n == 0), stop=(n == 5), perf_mode=DR,
                        )
                        n += 1
                if on_act:
                    nc.scalar.activation(qkT_sb[c3][:, sl], ps[:],
                                         Identity, bias=bqkv_sb[:, c3:c3 + 1])
                else:
                    nc.vector.tensor_scalar_add(
                        qkT_sb[c3][:, sl], ps[:], bqkv_sb[:, c3:c3 + 1],
                    )

            def emit_S_group(hp, segs, grp):
                qT = qkT_sb[hp]
                kT = qkT_sb[KC + hp]
                for h in (2 * hp, 2 * hp + 1):
                    poff = 64 * (h % 2)
                    S = ps_big.tile([128, 1024], fp32, tag="big", name="S")
                    span = 0
                    for j, base in grp:
                        qs = 128 * j
                        w = T - qs
                        span = base + w
                        first = base + min(512 - base % 512, w) if base < 512 \
                            else base + w
                        for a, b_ in ((base, first), (first, base + w)):
                            if b_ <= a:
                                continue
                            nc.tensor.matmul(
                                S[:, a:b_],
                                dr2(kT[poff:poff + 64, qs:qs + 128]),
                                dr2(qT[poff:poff + 64,
                                       qs + (a - base):qs + (b_ - base)]),
                                start=True, stop=True, perf_mode=DR,
                            )
                    P = pP.tile([128, 1024], f16, tag="P", name="P")
                    nc.scalar.activation(P[:, 0:span], S[:, 0:span], Exp,
                                         scale=2.0 ** -16)
                    for j, base in grp:
                        nc.gpsimd.tensor_mul(P[:, base:base + 128],
                                             P[:, base:base + 128],
                                             trimask_sb[:])
                        segs[h][j] = (P, base)

            def emit_S_j4_pair(hp, segs):
                # both heads' j4 block (512 cols each) share one PSUM tile
                # and one exp: halves the ACT op overhead for this group
                qT = qkT_sb[hp]
                kT = qkT_sb[KC + hp]
                S = ps_big.tile([128, 1024], fp32, tag="big", name="S")
                for idx, h in enumerate((2 * hp, 2 * hp + 1)):
                    poff = 64 * (h % 2)
                    nc.tensor.matmul(
                        S[:, 512 * idx:512 * idx + 512],
                        dr2(kT[poff:poff + 64, 512:640]),
                        dr2(qT[poff:poff + 64, 512:1024]),
                        start=True, stop=True, perf_mode=DR,
                    )
                P = pP.tile([128, 1024], f16, tag="P", name="P")
                nc.scalar.activation(P[:], S[:], Exp, scale=2.0 ** -16)
                for idx, h in enumerate((2 * hp, 2 * hp + 1)):
                    base = 512 * idx
                    nc.gpsimd.tensor_mul(P[:, base:base + 128],
                                         P[:, base:base + 128],
                                         trimask_sb[:])
                    segs[h][4] = (P, base)

            def emit_AV_half(hp, segs, yns, i, y2, half):
                pair = (2 * hp, 2 * hp + 1)
                b0 = 256 * half
                for idx, h in enumerate(pair):
                    c0 = b0 + 128 * idx
                    for j in range(i + 1):
                        P, base = segs[h][j]
                        off = base + 128 * (i - j)
                        nc.tensor.matmul(
                            y2[:, c0:c0 + D + 1],
                            P[:, off:off + 128],
                            v_sb[j][:, h, :],
                            start=(j == 0), stop=(j == i),
                        )
                recip = small.tile([128, 2], fp32, tag="recip", name="recip")
                nc.vector.reciprocal(
                    recip[:],
                    y2[:].rearrange("p (g c) -> p g c", c=128)[:, 2 * half:2 * half + 2, D],
                )
                # both heads' normalized y packed [128, 128] -> one transpose
                yn = small.tile([128, 2 * D], f16, tag="yn", name="yn",
                                bufs=10)
                for idx, h in enumerate(pair):
                    c0 = b0 + 128 * idx
                    nc.vector.tensor_scalar_mul(yn[:, idx * D:(idx + 1) * D],
                                                y2[:, c0:c0 + D],
                                                recip[:, idx:idx + 1])
                yns.append((i, yn))

            def emit_yT(hp, trs):
                nc.vector.tensor_scalar_add(
                    yT_sb[hp][:],
                    trs[:],
                    bqkv_sb[:, 2 * KC + hp:2 * KC + hp + 1],
                )

            def new_segs():
                return {h: {} for h in range(H)}

            # ---- cold start: qk(0) first, then S(0) groups (j0-first so
            # AV(0,0) unblocks early) with 1:1 v backfill ----
            segs = {0: new_segs(), 1: new_segs()}
            coldseq = ["sm"] * 8
            emit_qk_half(0, "q", 0, cold=coldseq[0])
            emit_qk_half(0, "k", 0, cold=coldseq[1])
            emit_qk_half(0, "q", 1, cold=coldseq[2])
            emit_S_group(0, segs[0], GROUPS[4])
            emit_qk_half(0, "k", 1, cold=coldseq[3])
            for n, (which, tchunk) in enumerate(
                    (("q", 0), ("q", 1), ("k", 0), ("k", 1))):
                emit_qk_half(1, which, tchunk, cold=coldseq[4 + n])
                if n < 3:
                    emit_S_group(0, segs[0], GROUPS[3 - n])
                else:
                    emit_S_j4_pair(0, segs[0])
            # v tiles before S(1): v's consumers (DVE copies) release big-
            # pool slots fast, while S tiles are released by ACT exps — v
            # first keeps the ring exp-decoupled and AV(0) unblocks early.
            # S(1)'s exps then keep ACT busy into pair 0 (depth-2 S/exp
            # pipeline: S(hp+2) is emitted during pair hp).
            emit_S_group(1, segs[1], GROUPS[4])
            emit_S_group(1, segs[1], GROUPS[3])
            emit_S_group(1, segs[1], GROUPS[2])
            emit_S_group(1, segs[1], GROUPS[1])
            emit_S_j4_pair(1, segs[1])
            for n in range(6):
                emit_v(n)

            # ---- pipelined cycles ----
            # per cycle: 8 AV i-steps; qk(hp+1) halves at steps 0,1,3; S(hp+1)
            # groups j0-first at steps 2,4,5,6,7 (j0 consumed first next cycle).
            qk_order = [("q", 0), ("q", 1), ("k", 0), ("k", 1)]
            def emit_transpose_slice(trs, yns, sl):
                for i, yn in yns[sl]:
                    nc.tensor.transpose(trs[:, 128 * i:128 * (i + 1)],
                                        yn[:], ident_sb[:])

            def emit_proj_mms(ps, cc, a, b_, kcs, stop_kc):
                for kc in kcs:
                    nc.tensor.matmul(
                        ps[:, a:b_],
                        wproj_sb[kc][:, cc * 128:(cc + 1) * 128],
                        yT_sb[kc][:, a:b_],
                        start=(kc == 0), stop=(kc == stop_kc),
                    )

            # staged projection partials: per cc, kc0-3 accumulated in a
            # transient big tile, then ACT-copied (+bias, fp32) to SBUF
            proj_part = {}
            proj_tmp = {}

            def unit_proj_half(cc, half):
                def f():
                    if half == 0:
                        proj_tmp[cc] = ps_big.tile([128, 1024], fp32,
                                                   tag="big", name="o_part_ps")
                    emit_proj_mms(proj_tmp[cc], cc, 512 * half,
                                  512 * (half + 1), range(3), None)
                    if half == 1:
                        part = osb.tile([128, T], f16, tag="o_part",
                                        name="o_part", bufs=6)
                        nc.scalar.activation(part[:], proj_tmp.pop(cc)[:],
                                             Identity,
                                             bias=bproj_sb[:, cc:cc + 1])
                        proj_part[cc] = part
                return f

            proj_units = [unit_proj_half(cc, half)
                          for cc in range(KC) for half in (0, 1)]

            prev_yns = None
            for hp in range(NPAIR):
                nxt = hp + 1 < NPAIR
                last = not nxt
                if hp + 2 < NPAIR:
                    segs[hp + 2] = new_segs()
                y2 = None
                yns = []
                trs = ps_tr.tile([128, 1024], f16, tag="tr", name="tr") \
                    if prev_yns is not None else None
                for i in range(TT):
                    if i % 2 == 0:
                        y2 = ps_sm.tile([128, 512], fp32, tag="sm",
                                        name="y2")
                    emit_AV_half(hp, segs[hp], yns, i, y2, i % 2)
                    if hp == 0 and i in (3, 5):
                        emit_v(6 if i == 3 else 7)
                    if prev_yns is not None:
                        emit_transpose_slice(trs, prev_yns,
                                             slice(i, i + 1))
                    if hp + 2 < NPAIR and i <= 3:
                        emit_qk_half(hp + 2, *qk_order[i])
                    if hp == 0:
                        gidx1 = {0: 4, 1: 3, 2: 2, 3: 1}.get(i)
                        if gidx1 is not None:
                            emit_S_group(1, segs[1], GROUPS[gidx1])
                        elif i == 4:
                            emit_S_j4_pair(1, segs[1])
                        gidx2 = {4: 4, 5: 3, 6: 2, 7: 1}.get(i)
                        if gidx2 is not None:
                            emit_S_group(2, segs[2], GROUPS[gidx2])
                    elif hp + 2 < NPAIR:
                        gidx = {3: 4, 4: 3, 5: 2, 6: 1}.get(i)
                        if gidx is not None:
                            emit_S_group(hp + 2, segs[hp + 2], GROUPS[gidx])
                        elif i == 7:
                            emit_S_j4_pair(hp + 2, segs[hp + 2])
                    elif hp == NPAIR - 2 and proj_units:
                        # pair 4 is qk/S-free: drain projection partials
                        # (kc 0-2; yT[0..2] final after pair 3) into the
                        # idle big pool
                        for u in (proj_units.pop(0) for _ in
                                  range(min(2, len(proj_units)))):
                            u()
                if prev_yns is not None:
                    emit_yT(hp - 1, trs)
                prev_yns = yns
                segs.pop(hp)
                if last:
                    while proj_units:
                        proj_units.pop(0)()
            trs = ps_tr.tile([128, 1024], f16, tag="tr", name="tr")
            hp5 = NPAIR - 1
            emit_transpose_slice(trs, prev_yns, slice(0, 4))
            nc.vector.tensor_scalar_add(
                yT_sb[hp5][:, 0:512], trs[:, 0:512],
                bqkv_sb[:, 2 * KC + hp5:2 * KC + hp5 + 1])
            emit_transpose_slice(trs, prev_yns, slice(4, 8))
            nc.vector.tensor_scalar_add(
                yT_sb[hp5][:, 512:1024], trs[:, 512:1024],
                bqkv_sb[:, 2 * KC + hp5:2 * KC + hp5 + 1])

            # ---- projection endgame: kc3-5 accumulation + DVE merge with
            # the staged bias-carrying kc0-2 partials ----
            for cc in range(KC):
                ps = ps_big.tile([128, 1024], fp32, tag="big", name="o_ps")
                for a, b_ in ((0, 512), (512, 1024)):
                    for kc in (3, 4, 5):
                        nc.tensor.matmul(
                            ps[:, a:b_],
                            wproj_sb[kc][:, cc * 128:(cc + 1) * 128],
                            yT_sb[kc][:, a:b_],
                            start=(kc == 3), stop=(kc == 5),
                        )
                o = osb.tile([128, T], f16, tag="o_sb", name="o_sb")
                nc.vector.tensor_add(o[:], ps[:], proj_part[cc][:])
                nc.sync.dma_start(out[cc * 128:(cc + 1) * 128, :], o[:])

    nc.compile()
    return nc


def _split_f8(a):
    hi = a.astype(_F8)
    lo = (a - hi.astype(np.float32)).astype(_F8)
    return np.ascontiguousarray(hi), np.ascontiguousarray(lo)


def _prep_inputs(x, w_qkv, b_qkv, w_proj, b_proj):
    # w scaled by 64 so fp8e4m3 quantization of the ~0.02-scale weights (and
    # their residuals) stays in the normal range; q/k biases scale to match
    # (exp scale folds the 64^2 back out); v descales at the on-chip copy.
    w64 = (w_qkv.astype(np.float32)) * 64.0
    wqkv_h, wqkv_l = _split_f8(w64)
    wqkv_8 = np.ascontiguousarray(np.concatenate([
        wqkv_h[:, 0:256], wqkv_l[:, 0:256],          # q01 h|l
        wqkv_h[:, C:C + 256], wqkv_l[:, C:C + 256],  # k01 h|l
        wqkv_h[:, 2 * C:], wqkv_l[:, 2 * C:],        # v h|l
        wqkv_h[:, 256:C], wqkv_l[:, 256:C],          # q25 h|l
        wqkv_h[:, C + 256:2 * C], wqkv_l[:, C + 256:2 * C],  # k25 h|l
    ], axis=1))
    wproj_f = np.ascontiguousarray(w_proj.astype(_F16))
    b_sc = b_qkv.astype(np.float32).copy()
    b_sc[:2 * C] *= 64.0
    bqkv_pc = np.ascontiguousarray(b_sc.reshape(C3 // 128, 128).T)
    bproj_pc = np.ascontiguousarray(
        b_proj.astype(np.float32).reshape(C // 128, 128).T)
    in_maps = []
    for b in range(B):
        xTb = np.ascontiguousarray(x[b].astype(np.float32).T)
        xh, xl = _split_f8(xTb)
        in_maps.append({
            "xT8": np.ascontiguousarray(np.stack([xh, xl], axis=1)),
            "wqkv8": wqkv_8,
            "wproj": wproj_f,
            "bqkv": bqkv_pc,
            "bproj": bproj_pc,
        })
    return in_maps


def _run(inputs, trace=False):
    from concourse.bass_utils import run_bass_kernel_spmd

    if "nc" not in _compiled:
        _compiled["nc"] = _build()
    nc = _compiled["nc"]
    in_maps = _prep_inputs(inputs["x"], inputs["w_qkv"], inputs["b_qkv"],
                           inputs["w_proj"], inputs["b_proj"])
    res = run_bass_kernel_spmd(nc, in_maps, list(range(B)), trace=trace)
    outs = np.stack([np.asarray(res.results[b]["out"]).T for b in range(B)])
    return outs.astype(np.float32), res


def kernel(x, w_qkv, b_qkv, w_proj, b_proj):
    outs, _ = _run(dict(x=x, w_qkv=w_qkv, b_qkv=b_qkv,
                        w_proj=w_proj, b_proj=b_proj))
    return outs


# revision 32
# speedup vs baseline: 1.2250x; 1.0169x over previous
"""Causal self-attention (B=8, T=1024, C=768, H=12, D=64) on 8 TRN2 NeuronCores.

Sharding: data-parallel over batch — core b handles batch element b. No
collectives. Host pre-transposes x to x^T[b] and pre-casts operands to fp16;
matmuls run fp16 with fp32 PSUM accumulation, except S = K Q^T which runs
fp8e4m3 in DoubleRow perf mode (0.5 cycles/row): both operands present their
single 64-channel contraction plane twice via a stride-0 broadcast dim, so
the DoubleRow two-plane sum computes exactly 2*S, folded into the exp scale
(1/16 instead of 1/8).

Per-core algorithm:
  v = x Wv in [t, c] layout (x^T stationary); v bias folded into the y^T
  stage (exact: softmax rows sum to 1). q^T,k^T = (Wqkv^T x^T + b) in
  [c3, t] layout (weights stationary), quantized to fp8e4m3 at the bias add.
  Per head h, key-block j (128 keys): S^T = K_j Q^T in PSUM [keys, q]
  (causal: only q >= 128j columns; blocks pack into [128,1024] PSUM tiles as
  {j0},{j1,j7},{j2,j6},{j3,j5},{j4} so one ACT exp covers each tile,
  scale=1/16), triangular mask-multiply on diagonal 128x128 blocks. Per
  q-tile i: y'[q,65] = sum_j P_j^T.T @ [V_j|1] accumulated in PSUM (two
  heads x two i-steps share one PSUM bank); col 64 is the softmax
  denominator. Normalize via per-partition reciprocal+scale, PE-transpose
  into a per-pair [128, 1024] fp16 PSUM strip (head parity in partition
  halves), one DVE pass per pair adds the v-bias and lands y^T in SBUF.
  out[t, c] = y^T.T @ Wproj + b_proj.

Emission is software-pipelined (static per-engine order => head-of-line
blocking): cycle hp interleaves AV(hp) i-steps with qk(hp+1) half-chunks
and S(hp+1) groups so PE fills ACT-paced exp stalls; v tiles fill the S(0)
cold start.

PSUM budget (8 banks): big [128,1024]fp32 x2 (4, shared v/S/o) +
shared [128,512]fp32 x3 (3, qk halves + y' two-heads-x-two-i-steps) +
tr [128,1024]fp16 x1 (1).
"""

import ml_dtypes
import numpy as np

B, T, C = 8, 1024, 768
H, D = 12, 64
C3 = 3 * C
KC = C // 128          # 6 contraction chunks over c_in
TT = T // 128          # 8 t-tiles of 128
NPAIR = H // 2

BIG_BUFS = 2
QK_BUFS = 1
SM_BUFS = 2
TR_BUFS = 1
PP_BUFS = 32           # 10 P segs/pair x 3 pairs in flight + slack

_F16 = np.float16
_F8 = ml_dtypes.float8_e4m3

_compiled = {}


def _build():
    from concourse import bacc, mybir
    import concourse.tile as tile
    from concourse.masks import make_identity, make_upper_triangular

    fp32 = mybir.dt.float32
    f16 = mybir.dt.float16
    f8 = mybir.dt.float8e4
    DR = mybir.MatmulPerfMode.DoubleRow

    nc = bacc.Bacc("TRN2", target_bir_lowering=False, debug=False,
                   enable_asserts=True, num_devices=B)

    # x: hi/lo fp8 planes interleaved; w: hi/lo in load-order column blocks
    # [q01h q01l k01h k01l vh vl q25h q25l k25h k25l] so each cold DMA is one
    # contiguous 3D copy
    xT8 = nc.dram_tensor("xT8", [C, 2, T], f8, kind="ExternalInput")
    wqkv8 = nc.dram_tensor("wqkv8", [C, 2 * C3], f8, kind="ExternalInput")
    wproj = nc.dram_tensor("wproj", [C, C], f16, kind="ExternalInput")
    # b_qkv rearranged host-side to [128, 18]: col j holds b_qkv[128j:128j+128]
    bqkv = nc.dram_tensor("bqkv", [128, C3 // 128], fp32, kind="ExternalInput")
    # b_proj rearranged host-side to [128, 6]: col j holds b_proj[128j:128j+128]
    bproj = nc.dram_tensor("bproj", [128, C // 128], fp32, kind="ExternalInput")
    # out is produced transposed [C, T] in f16; host transposes + casts back
    out = nc.dram_tensor("out", [C, T], f16, kind="ExternalOutput")

    Exp = mybir.ActivationFunctionType.Exp
    # S-block packing: groups of (j, base column) sharing one [128,1024]
    # PSUM tile => one exp per tile. Bases keep each block inside the tile.
    GROUPS = [((4, 0),), ((3, 0), (5, 640)), ((2, 0), (6, 768)),
              ((1, 0), (7, 896)), ((0, 0),)]

    def dr2(ap):
        # present a 2D AP as [p, 2, n] with a stride-0 plane dim: DoubleRow
        # then sums the same plane twice => computes 2x the matmul.
        p, n = ap.shape
        return ap.rearrange("p (o t) -> p o t", o=1).to_broadcast([p, 2, n])

    with tile.TileContext(nc) as tc:
        with (
            tc.tile_pool(name="const", bufs=1) as const,
            tc.tile_pool(name="pP", bufs=PP_BUFS) as pP,
            tc.tile_pool(name="small", bufs=6) as small,
            tc.tile_pool(name="osb", bufs=4) as osb,
            tc.tile_pool(name="ps_big", bufs=BIG_BUFS, space="PSUM") as ps_big,
            tc.tile_pool(name="ps_qk", bufs=QK_BUFS, space="PSUM") as ps_qk,
            tc.tile_pool(name="ps_sm", bufs=SM_BUFS, space="PSUM") as ps_sm,
            tc.tile_pool(name="ps_tr", bufs=TR_BUFS, space="PSUM") as ps_tr,
        ):
            # ---- persistent SBUF loads ----
            # All input streaming via the two HWDGE queues (SP + ACT), in
            # need-order: hp0+1 q/k sliver cols + x^T first (qk(0)), then
            # v cols, hp2-5 q/k cols, wproj.  DMA_ENGINES is a serial
            # resource: issue order IS the arrival order.
            bqkv_sb = const.tile([128, C3 // 128], fp32, tag="bqkv", name="bqkv")
            w8_sb = const.tile([128, KC, 2 * C3], f8, tag="w8", name="w8")
            w_src = wqkv8.rearrange("(k p) c -> p k c", k=KC)

            def wcol(hl, c3):
                # load-order flat column of sliver c3's first column
                if c3 < 2:
                    return 256 * hl + c3 * 128
                if 6 <= c3 < 8:
                    return 512 + 256 * hl + (c3 - 6) * 128
                if 2 <= c3 < 6:
                    return 2560 + 512 * hl + (c3 - 2) * 128
                return 3584 + 512 * hl + (c3 - 8) * 128

            WV = [1024, 1792]  # v-section bases (hi, lo)
            x8_sb = const.tile([128, KC, 2, T], f8, tag="x8", name="x8")
            x_src = xT8.rearrange("(k p) two t -> p k two t", k=KC)
            # ALL input DMAs on the SP queue in exact need-order (HWDGE
            # generation and the DMA engine track are serial resources, and
            # scalar-queue DMAs would eat ACT SEQ time): S(0)-j0's inputs
            # first (x8h both halves + q01/k01 slivers), then v weights +
            # x8l, then pair-2+ slivers, wproj.
            nc.sync.dma_start(bqkv_sb[:], bqkv[:, :])
            nc.sync.dma_start(x8_sb[:, :, 0, 0:512], x_src[:, :, 0, 0:512])
            nc.sync.dma_start(w8_sb[:, :, 0:512], w_src[:, :, 0:512])
            nc.sync.dma_start(x8_sb[:, :, 0, 512:1024],
                              x_src[:, :, 0, 512:1024])
            nc.sync.dma_start(w8_sb[:, :, 512:1024], w_src[:, :, 512:1024])
            nc.sync.dma_start(w8_sb[:, :, 1024:2560], w_src[:, :, 1024:2560])
            nc.sync.dma_start(x8_sb[:, :, 1, 0:512], x_src[:, :, 1, 0:512])
            nc.sync.dma_start(x8_sb[:, :, 1, 512:1024],
                              x_src[:, :, 1, 512:1024])
            nc.sync.dma_start(w8_sb[:, :, 2560:3584], w_src[:, :, 2560:3584])
            nc.sync.dma_start(w8_sb[:, :, 3584:4608], w_src[:, :, 3584:4608])
            bproj_sb = const.tile([128, C // 128], fp32, tag="bproj", name="bproj")
            nc.sync.dma_start(bproj_sb[:], bproj[:, :])
            wproj_big = const.tile([128, KC, C], f16, tag="wproj", name="wproj")
            wproj_sb = [wproj_big[:, kc] for kc in range(KC)]
            nc.sync.dma_start(
                wproj_big[:],
                wproj.rearrange("(k p) c -> p k c", k=KC),
            )
            ident_sb = const.tile([128, 128], f16, tag="ident", name="ident")
            make_identity(nc, ident_sb[:])
            # keep columns m >= l (query >= key) on the diagonal block
            trimask_sb = const.tile([128, 128], f16, tag="trimask", name="trimask")
            make_upper_triangular(nc, trimask_sb[:], val=1.0, diag=True)

            qkT_sb = [const.tile([128, T], f8, tag=f"qkT{c3}", name=f"qkT{c3}")
                      for c3 in range(2 * KC)]
            # v packed [t, 12 heads x (64 + ones col)]
            v_sb = [const.tile([128, H, D + 1], f16, tag=f"v{tt}", name=f"v{tt}")
                    for tt in range(TT)]
            yT_sb = [const.tile([128, T], f16, tag=f"yT{kc}", name=f"yT{kc}")
                     for kc in range(KC)]

            def emit_v(tt):
                # two sm-pool tiles (fast DVE-only consumers) keep v off the
                # exp-paced big ring
                psA = ps_sm.tile([128, 512], fp32, tag="sm", name="vA_ps")
                psB = ps_sm.tile([128, 512], fp32, tag="sm", name="vB_ps")
                tsl = slice(tt * 128, (tt + 1) * 128)
                terms = [(0, 0), (0, 1), (1, 0)]
                # x8l term last so v matmuls start before x8l lands
                n = 0
                for xs, ws in terms:
                    vb = WV[ws]
                    for cp in range(KC // 2):
                        kk = slice(2 * cp, 2 * cp + 2)
                        nc.tensor.matmul(
                            psA[:],
                            x8_sb[:, kk, xs, tsl],
                            w8_sb[:, kk, vb:vb + 512],
                            start=(n == 0), stop=(n == 8), perf_mode=DR,
                        )
                        nc.tensor.matmul(
                            psB[:, 0:256],
                            x8_sb[:, kk, xs, tsl],
                            w8_sb[:, kk, vb + 512:vb + 768],
                            start=(n == 0), stop=(n == 8), perf_mode=DR,
                        )
                        n += 1
                vv = v_sb[tt]
                nc.vector.tensor_scalar_mul(
                    vv[:, 0:8, 0:D],
                    psA[:].rearrange("p (h d) -> p h d", d=D),
                    1.0 / 64.0,
                )
                nc.vector.tensor_scalar_mul(
                    vv[:, 8:12, 0:D],
                    psB[:, 0:256].rearrange("p (h d) -> p h d", d=D),
                    1.0 / 64.0,
                )
                nc.vector.memset(vv[:, :, D:D + 1], 1.0)

            Identity = mybir.ActivationFunctionType.Identity

            def emit_qk_half(hp, which, tchunk, on_act=False, cold=False):
                c3 = hp if which == "q" else KC + hp
                # cold-start halves alternate the y2 pool (idle until pair
                # 0) with the dedicated bank: an effective ring-3 while DMA
                # streams
                if cold == "sm":
                    pool, tg = ps_sm, "sm"
                else:
                    pool, tg = ps_qk, "qk"
                ps = pool.tile([128, 512], fp32, tag=tg, name="qk_ps")
                sl = slice(tchunk * 512, (tchunk + 1) * 512)
                n = 0
                for ws in (0, 1):
                    wc = wcol(ws, c3)
                    for cp in range(KC // 2):
                        kk = slice(2 * cp, 2 * cp + 2)
                        nc.tensor.matmul(
                            ps[:],
                            w8_sb[:, kk, wc:wc + 128],
                            x8_sb[:, kk, 0, sl],
                            start=(n == 0), stop=(n == 5), perf_mode=DR,
                        )
                        n += 1
                if on_act:
                    nc.scalar.activation(qkT_sb[c3][:, sl], ps[:],
                                         Identity, bias=bqkv_sb[:, c3:c3 + 1])
                else:
                    nc.vector.tensor_scalar_add(
                        qkT_sb[c3][:, sl], ps[:], bqkv_sb[:, c3:c3 + 1],
                    )

            def emit_S_group(hp, segs, grp):
                qT = qkT_sb[hp]
                kT = qkT_sb[KC + hp]
                for h in (2 * hp, 2 * hp + 1):
                    poff = 64 * (h % 2)
                    S = ps_big.tile([128, 1024], fp32, tag="big", name="S")
                    span = 0
                    for j, base in grp:
                        qs = 128 * j
                        w = T - qs
                        span = base + w
                        first = base + min(512 - base % 512, w) if base < 512 \
                            else base + w
                        for a, b_ in ((base, first), (first, base + w)):
                            if b_ <= a:
                                continue
                            nc.tensor.matmul(
                                S[:, a:b_],
                                dr2(kT[poff:poff + 64, qs:qs + 128]),
                                dr2(qT[poff:poff + 64,
                                       qs + (a - base):qs + (b_ - base)]),
                                start=True, stop=True, perf_mode=DR,
                            )
                    P = pP.tile([128, 1024], f16, tag="P", name="P")
                    nc.scalar.activation(P[:, 0:span], S[:, 0:span], Exp,
                                         scale=2.0 ** -16)
                    for j, base in grp:
                        nc.gpsimd.tensor_mul(P[:, base:base + 128],
                                             P[:, base:base + 128],
                                             trimask_sb[:])
                        segs[h][j] = (P, base)

            def emit_S_j4_pair(hp, segs):
                # both heads' j4 block (512 cols each) share one PSUM tile
                # and one exp: halves the ACT op overhead for this group
                qT = qkT_sb[hp]
                kT = qkT_sb[KC + hp]
                S = ps_big.tile([128, 1024], fp32, tag="big", name="S")
                for idx, h in enumerate((2 * hp, 2 * hp + 1)):
                    poff = 64 * (h % 2)
                    nc.tensor.matmul(
                        S[:, 512 * idx:512 * idx + 512],
                        dr2(kT[poff:poff + 64, 512:640]),
                        dr2(qT[poff:poff + 64, 512:1024]),
                        start=True, stop=True, perf_mode=DR,
                    )
                P = pP.tile([128, 1024], f16, tag="P", name="P")
                nc.scalar.activation(P[:], S[:], Exp, scale=2.0 ** -16)
                for idx, h in enumerate((2 * hp, 2 * hp + 1)):
                    base = 512 * idx
                    nc.gpsimd.tensor_mul(P[:, base:base + 128],
                                         P[:, base:base + 128],
                                         trimask_sb[:])
                    segs[h][4] = (P, base)

            def emit_AV_half(hp, segs, yns, i, y2, half):
                pair = (2 * hp, 2 * hp + 1)
                b0 = 256 * half
                for idx, h in enumerate(pair):
                    c0 = b0 + 128 * idx
                    for j in range(i + 1):
                        P, base = segs[h][j]
                        off = base + 128 * (i - j)
                        nc.tensor.matmul(
                            y2[:, c0:c0 + D + 1],
                            P[:, off:off + 128],
                            v_sb[j][:, h, :],
                            start=(j == 0), stop=(j == i),
                        )
                recip = small.tile([128, 2], fp32, tag="recip", name="recip")
                nc.vector.reciprocal(
                    recip[:],
                    y2[:].rearrange("p (g c) -> p g c", c=128)[:, 2 * half:2 * half + 2, D],
                )
                # both heads' normalized y packed [128, 128] -> one transpose
                yn = small.tile([128, 2 * D], f16, tag="yn", name="yn",
                                bufs=10)
                for idx, h in enumerate(pair):
                    c0 = b0 + 128 * idx
                    nc.vector.tensor_scalar_mul(yn[:, idx * D:(idx + 1) * D],
                                                y2[:, c0:c0 + D],
                                                recip[:, idx:idx + 1])
                yns.append((i, yn))

            def emit_yT(hp, trs):
                nc.vector.tensor_scalar_add(
                    yT_sb[hp][:],
                    trs[:],
                    bqkv_sb[:, 2 * KC + hp:2 * KC + hp + 1],
                )

            def new_segs():
                return {h: {} for h in range(H)}

            # ---- cold start: qk(0) first, then S(0) groups (j0-first so
            # AV(0,0) unblocks early) with 1:1 v backfill ----
            segs = {0: new_segs(), 1: new_segs()}
            coldseq = ["sm"] * 8
            emit_qk_half(0, "q", 0, cold=coldseq[0])
            emit_qk_half(0, "k", 0, cold=coldseq[1])
            emit_qk_half(0, "q", 1, cold=coldseq[2])
            emit_S_group(0, segs[0], GROUPS[4])
            emit_qk_half(0, "k", 1, cold=coldseq[3])
            for n, (which, tchunk) in enumerate(
                    (("q", 0), ("q", 1), ("k", 0), ("k", 1))):
                emit_qk_half(1, which, tchunk, cold=coldseq[4 + n])
                if n < 3:
                    emit_S_group(0, segs[0], GROUPS[3 - n])
                else:
                    emit_S_j4_pair(0, segs[0])
            # v tiles before S(1): v's consumers (DVE copies) release big-
            # pool slots fast, while S tiles are released by ACT exps — v
            # first keeps the ring exp-decoupled and AV(0) unblocks early.
            # S(1)'s exps then keep ACT busy into pair 0 (depth-2 S/exp
            # pipeline: S(hp+2) is emitted during pair hp).
            emit_S_group(1, segs[1], GROUPS[4])
            emit_S_group(1, segs[1], GROUPS[3])
            emit_S_group(1, segs[1], GROUPS[2])
            emit_S_group(1, segs[1], GROUPS[1])
            emit_S_j4_pair(1, segs[1])
            for n in range(6):
                emit_v(n)

            # ---- pipelined cycles ----
            # per cycle: 8 AV i-steps; qk(hp+1) halves at steps 0,1,3; S(hp+1)
            # groups j0-first at steps 2,4,5,6,7 (j0 consumed first next cycle).
            qk_order = [("q", 0), ("q", 1), ("k", 0), ("k", 1)]
            def emit_transpose_slice(trs, yns, sl):
                for i, yn in yns[sl]:
                    nc.tensor.transpose(trs[:, 128 * i:128 * (i + 1)],
                                        yn[:], ident_sb[:])

            def emit_proj_mms(ps, cc, a, b_, kcs, stop_kc):
                for kc in kcs:
                    nc.tensor.matmul(
                        ps[:, a:b_],
                        wproj_sb[kc][:, cc * 128:(cc + 1) * 128],
                        yT_sb[kc][:, a:b_],
                        start=(kc == 0), stop=(kc == stop_kc),
                    )

            # staged projection partials: per cc, kc0-3 accumulated in a
            # transient big tile, then ACT-copied (+bias, fp32) to SBUF
            proj_part = {}
            proj_tmp = {}

            def unit_proj_half(cc, half):
                def f():
                    if half == 0:
                        proj_tmp[cc] = ps_big.tile([128, 1024], fp32,
                                                   tag="big", name="o_part_ps")
                    emit_proj_mms(proj_tmp[cc], cc, 512 * half,
                                  512 * (half + 1), range(3), None)
                    if half == 1:
                        part = osb.tile([128, T], f16, tag="o_part",
                                        name="o_part", bufs=6)
                        nc.scalar.activation(part[:], proj_tmp.pop(cc)[:],
                                             Identity,
                                             bias=bproj_sb[:, cc:cc + 1])
                        proj_part[cc] = part
                return f

            proj_units = [unit_proj_half(cc, half)
                          for cc in range(KC) for half in (0, 1)]

            o_sb_tiles = {}

            def unit_endgame(cc, half, slot):
                a, b_ = 512 * half, 512 * (half + 1)

                def f():
                    # kc3-5 accumulation in a [128,512] slot + DVE merge
                    # with the staged kc0-2 partial; DMA per half
                    pool, tg = (ps_sm, "sm") if slot else (ps_qk, "qk")
                    ps = pool.tile([128, 512], fp32, tag=tg, name="eg_ps")
                    for kc in (3, 4, 5):
                        nc.tensor.matmul(
                            ps[:],
                            wproj_sb[kc][:, cc * 128:(cc + 1) * 128],
                            yT_sb[kc][:, a:b_],
                            start=(kc == 3), stop=(kc == 5),
                        )
                    if cc not in o_sb_tiles:
                        o_sb_tiles[cc] = osb.tile([128, T], f16, tag="o_sb",
                                                  name="o_sb")
                    o = o_sb_tiles[cc]
                    nc.vector.tensor_add(o[:, a:b_], ps[:],
                                         proj_part[cc][:, a:b_])
                    nc.sync.dma_start(out[cc * 128:(cc + 1) * 128, a:b_],
                                      o[:, a:b_])
                return f

            endgame_units = [unit_endgame(cc, half, (cc * 2 + half) % 3 != 2)
                             for half in (0, 1) for cc in range(KC)]

            prev_yns = None
            for hp in range(NPAIR):
                nxt = hp + 1 < NPAIR
                last = not nxt
                if hp + 2 < NPAIR:
                    segs[hp + 2] = new_segs()
                y2 = None
                yns = []
                trs = ps_tr.tile([128, 1024], f16, tag="tr", name="tr") \
                    if prev_yns is not None else None
                for i in range(TT):
                    if i % 2 == 0:
                        y2 = ps_sm.tile([128, 512], fp32, tag="sm",
                                        name="y2")
                    emit_AV_half(hp, segs[hp], yns, i, y2, i % 2)
                    if hp == 0 and i in (3, 5):
                        emit_v(6 if i == 3 else 7)
                    if prev_yns is not None and not last:
                        emit_transpose_slice(trs, prev_yns,
                                             slice(i, i + 1))
                    elif last:
                        # final pair: drain pair-4 transposes early (2/step),
                        # then transpose own yns in-pair so yT[5] halves land
                        # mid-pair and the projection endgame overlaps
                        if i <= 3:
                            emit_transpose_slice(trs, prev_yns,
                                                 slice(2 * i, 2 * i + 2))
                            if i == 3:
                                emit_yT(hp - 1, trs)
                        else:
                            if i == 4:
                                trs5 = ps_tr.tile([128, 1024], f16,
                                                  tag="tr", name="tr5")
                                emit_transpose_slice(trs5, yns, slice(0, 3))
                            elif i == 5:
                                emit_transpose_slice(trs5, yns, slice(3, 4))
                                nc.vector.tensor_scalar_add(
                                    yT_sb[hp][:, 0:512], trs5[:, 0:512],
                                    bqkv_sb[:, 2 * KC + hp:2 * KC + hp + 1])
                            else:
                                emit_transpose_slice(trs5, yns,
                                                     slice(i - 2, i - 1))
                                for u in (endgame_units.pop(0) for _ in
                                          range(min(3, len(endgame_units)))):
                                    u()
                    if hp + 2 < NPAIR and i <= 3:
                        emit_qk_half(hp + 2, *qk_order[i])
                    if hp == 0:
                        gidx1 = {0: 4, 1: 3, 2: 2, 3: 1}.get(i)
                        if gidx1 is not None:
                            emit_S_group(1, segs[1], GROUPS[gidx1])
                        elif i == 4:
                            emit_S_j4_pair(1, segs[1])
                        gidx2 = {4: 4, 5: 3, 6: 2, 7: 1}.get(i)
                        if gidx2 is not None:
                            emit_S_group(2, segs[2], GROUPS[gidx2])
                    elif hp + 2 < NPAIR:
                        gidx = {3: 4, 4: 3, 5: 2, 6: 1}.get(i)
                        if gidx is not None:
                            emit_S_group(hp + 2, segs[hp + 2], GROUPS[gidx])
                        elif i == 7:
                            emit_S_j4_pair(hp + 2, segs[hp + 2])
                    elif hp == NPAIR - 2 and proj_units:
                        # pair 4 is qk/S-free: drain projection partials
                        # (kc 0-2; yT[0..2] final after pair 3) into the
                        # idle big pool
                        for u in (proj_units.pop(0) for _ in
                                  range(min(2, len(proj_units)))):
                            u()
                if hp == 0:
                    emit_S_j4_pair(2, segs[2])
                if prev_yns is not None and not last:
                    emit_yT(hp - 1, trs)
                prev_yns = yns
                segs.pop(hp)
                if last:
                    while proj_units:
                        proj_units.pop(0)()
            hp5 = NPAIR - 1
            emit_transpose_slice(trs5, prev_yns, slice(6, 8))
            nc.vector.tensor_scalar_add(
                yT_sb[hp5][:, 512:1024], trs5[:, 512:1024],
                bqkv_sb[:, 2 * KC + hp5:2 * KC + hp5 + 1])
            while endgame_units:
                endgame_units.pop(0)()

    nc.compile()
    return nc


def _split_f8(a):
    hi = a.astype(_F8)
    lo = (a - hi.astype(np.float32)).astype(_F8)
    return np.ascontiguousarray(hi), np.ascontiguousarray(lo)


def _prep_inputs(x, w_qkv, b_qkv, w_proj, b_proj):
    # w scaled by 64 so fp8e4m3 quantization of the ~0.02-scale weights (and
    # their residuals) stays in the normal range; q/k biases scale to match
    # (exp scale folds the 64^2 back out); v descales at the on-chip copy.
    w64 = (w_qkv.astype(np.float32)) * 64.0
    wqkv_h, wqkv_l = _split_f8(w64)
    wqkv_8 = np.ascontiguousarray(np.concatenate([
        wqkv_h[:, 0:256], wqkv_l[:, 0:256],          # q01 h|l
        wqkv_h[:, C:C + 256], wqkv_l[:, C:C + 256],  # k01 h|l
        wqkv_h[:, 2 * C:], wqkv_l[:, 2 * C:],        # v h|l
        wqkv_h[:, 256:C], wqkv_l[:, 256:C],          # q25 h|l
        wqkv_h[:, C + 256:2 * C], wqkv_l[:, C + 256:2 * C],  # k25 h|l
    ], axis=1))
    wproj_f = np.ascontiguousarray(w_proj.astype(_F16))
    b_sc = b_qkv.astype(np.float32).copy()
    b_sc[:2 * C] *= 64.0
    bqkv_pc = np.ascontiguousarray(b_sc.reshape(C3 // 128, 128).T)
    bproj_pc = np.ascontiguousarray(
        b_proj.astype(np.float32).reshape(C // 128, 128).T)
    in_maps = []
    for b in range(B):
        xTb = np.ascontiguousarray(x[b].astype(np.float32).T)
        xh, xl = _split_f8(xTb)
        in_maps.append({
            "xT8": np.ascontiguousarray(np.stack([xh, xl], axis=1)),
            "wqkv8": wqkv_8,
            "wproj": wproj_f,
            "bqkv": bqkv_pc,
            "bproj": bproj_pc,
        })
    return in_maps


def _run(inputs, trace=False):
    from concourse.bass_utils import run_bass_kernel_spmd

    if "nc" not in _compiled:
        _compiled["nc"] = _build()
    nc = _compiled["nc"]
    in_maps = _prep_inputs(inputs["x"], inputs["w_qkv"], inputs["b_qkv"],
                           inputs["w_proj"], inputs["b_proj"])
    res = run_bass_kernel_spmd(nc, in_maps, list(range(B)), trace=trace)
    outs = np.stack([np.asarray(res.results[b]["out"]).T for b in range(B)])
    return outs.astype(np.float32), res


def kernel(x, w_qkv, b_qkv, w_proj, b_proj):
    outs, _ = _run(dict(x=x, w_qkv=w_qkv, b_qkv=b_qkv,
                        w_proj=w_proj, b_proj=b_proj))
    return outs


# revision 39
# speedup vs baseline: 1.2500x; 1.0204x over previous
"""Causal self-attention (B=8, T=1024, C=768, H=12, D=64) on 8 TRN2 NeuronCores.

Sharding: data-parallel over batch — core b handles batch element b. No
collectives. Host pre-transposes x to x^T[b] and pre-casts operands to fp16;
matmuls run fp16 with fp32 PSUM accumulation, except S = K Q^T which runs
fp8e4m3 in DoubleRow perf mode (0.5 cycles/row): both operands present their
single 64-channel contraction plane twice via a stride-0 broadcast dim, so
the DoubleRow two-plane sum computes exactly 2*S, folded into the exp scale
(1/16 instead of 1/8).

Per-core algorithm:
  v = x Wv in [t, c] layout (x^T stationary); v bias folded into the y^T
  stage (exact: softmax rows sum to 1). q^T,k^T = (Wqkv^T x^T + b) in
  [c3, t] layout (weights stationary), quantized to fp8e4m3 at the bias add.
  Per head h, key-block j (128 keys): S^T = K_j Q^T in PSUM [keys, q]
  (causal: only q >= 128j columns; blocks pack into [128,1024] PSUM tiles as
  {j0},{j1,j7},{j2,j6},{j3,j5},{j4} so one ACT exp covers each tile,
  scale=1/16), triangular mask-multiply on diagonal 128x128 blocks. Per
  q-tile i: y'[q,65] = sum_j P_j^T.T @ [V_j|1] accumulated in PSUM (two
  heads x two i-steps share one PSUM bank); col 64 is the softmax
  denominator. Normalize via per-partition reciprocal+scale, PE-transpose
  into a per-pair [128, 1024] fp16 PSUM strip (head parity in partition
  halves), one DVE pass per pair adds the v-bias and lands y^T in SBUF.
  out[t, c] = y^T.T @ Wproj + b_proj.

Emission is software-pipelined (static per-engine order => head-of-line
blocking): cycle hp interleaves AV(hp) i-steps with qk(hp+1) half-chunks
and S(hp+1) groups so PE fills ACT-paced exp stalls; v tiles fill the S(0)
cold start.

PSUM budget (8 banks): big [128,1024]fp32 x2 (4, shared v/S/o) +
shared [128,512]fp32 x3 (3, qk halves + y' two-heads-x-two-i-steps) +
tr [128,1024]fp16 x1 (1).
"""

import ml_dtypes
import numpy as np

B, T, C = 8, 1024, 768
H, D = 12, 64
C3 = 3 * C
KC = C // 128          # 6 contraction chunks over c_in
TT = T // 128          # 8 t-tiles of 128
NPAIR = H // 2

BIG_BUFS = 2
QK_BUFS = 1
SM_BUFS = 2
TR_BUFS = 1
PP_BUFS = 32           # 10 P segs/pair x 3 pairs in flight + slack

_F16 = np.float16
_F8 = ml_dtypes.float8_e4m3

_compiled = {}


def _build():
    from concourse import bacc, mybir
    import concourse.tile as tile
    from concourse.masks import make_identity, make_upper_triangular

    fp32 = mybir.dt.float32
    f16 = mybir.dt.float16
    f8 = mybir.dt.float8e4
    DR = mybir.MatmulPerfMode.DoubleRow

    nc = bacc.Bacc("TRN2", target_bir_lowering=False, debug=False,
                   enable_asserts=True, num_devices=B)

    # x: hi/lo fp8 planes interleaved; w: hi/lo in load-order column blocks
    # [q01h q01l k01h k01l vh vl q25h q25l k25h k25l] so each cold DMA is one
    # contiguous 3D copy
    xT8 = nc.dram_tensor("xT8", [C, 2, T], f8, kind="ExternalInput")
    wqkv8 = nc.dram_tensor("wqkv8", [C, 2 * C3], f8, kind="ExternalInput")
    wproj = nc.dram_tensor("wproj", [C, C], f16, kind="ExternalInput")
    # b_qkv rearranged host-side to [128, 18]: col j holds b_qkv[128j:128j+128]
    bqkv = nc.dram_tensor("bqkv", [128, C3 // 128], fp32, kind="ExternalInput")
    # b_proj rearranged host-side to [128, 6]: col j holds b_proj[128j:128j+128]
    bproj = nc.dram_tensor("bproj", [128, C // 128], fp32, kind="ExternalInput")
    # out is produced transposed [C, T] in f16; host transposes + casts back
    out = nc.dram_tensor("out", [C, T], f16, kind="ExternalOutput")

    Exp = mybir.ActivationFunctionType.Exp
    # S-block packing: groups of (j, base column) sharing one [128,1024]
    # PSUM tile => one exp per tile. Bases keep each block inside the tile.
    GROUPS = [((4, 0),), ((3, 0), (5, 640)), ((2, 0), (6, 768)),
              ((1, 0), (7, 896)), ((0, 0),)]

    def dr2(ap):
        # present a 2D AP as [p, 2, n] with a stride-0 plane dim: DoubleRow
        # then sums the same plane twice => computes 2x the matmul.
        p, n = ap.shape
        return ap.rearrange("p (o t) -> p o t", o=1).to_broadcast([p, 2, n])

    with tile.TileContext(nc) as tc:
        with (
            tc.tile_pool(name="const", bufs=1) as const,
            tc.tile_pool(name="pP", bufs=PP_BUFS) as pP,
            tc.tile_pool(name="small", bufs=6) as small,
            tc.tile_pool(name="osb", bufs=4) as osb,
            tc.tile_pool(name="ps_big", bufs=BIG_BUFS, space="PSUM") as ps_big,
            tc.tile_pool(name="ps_qk", bufs=QK_BUFS, space="PSUM") as ps_qk,
            tc.tile_pool(name="ps_sm", bufs=SM_BUFS, space="PSUM") as ps_sm,
            tc.tile_pool(name="ps_tr", bufs=TR_BUFS, space="PSUM") as ps_tr,
        ):
            # ---- persistent SBUF loads ----
            # All input streaming via the two HWDGE queues (SP + ACT), in
            # need-order: hp0+1 q/k sliver cols + x^T first (qk(0)), then
            # v cols, hp2-5 q/k cols, wproj.  DMA_ENGINES is a serial
            # resource: issue order IS the arrival order.
            bqkv_sb = const.tile([128, C3 // 128], fp32, tag="bqkv", name="bqkv")
            w8_sb = const.tile([128, KC, 2 * C3], f8, tag="w8", name="w8")
            w_src = wqkv8.rearrange("(k p) c -> p k c", k=KC)

            def wcol(hl, c3):
                # load-order flat column of sliver c3's first column
                if c3 < 2:
                    return 256 * hl + c3 * 128
                if 6 <= c3 < 8:
                    return 512 + 256 * hl + (c3 - 6) * 128
                if 2 <= c3 < 6:
                    return 2560 + 512 * hl + (c3 - 2) * 128
                return 3584 + 512 * hl + (c3 - 8) * 128

            WV = [1024, 1792]  # v-section bases (hi, lo)
            x8_sb = const.tile([128, KC, 2, T], f8, tag="x8", name="x8")
            x_src = xT8.rearrange("(k p) two t -> p k two t", k=KC)
            # ALL input DMAs on the SP queue in exact need-order (HWDGE
            # generation and the DMA engine track are serial resources, and
            # scalar-queue DMAs would eat ACT SEQ time): S(0)-j0's inputs
            # first (x8h both halves + q01/k01 slivers), then v weights +
            # x8l, then pair-2+ slivers, wproj.
            nc.sync.dma_start(bqkv_sb[:], bqkv[:, :])
            nc.sync.dma_start(x8_sb[:, :, 0, 0:512], x_src[:, :, 0, 0:512])
            nc.sync.dma_start(w8_sb[:, :, 0:512], w_src[:, :, 0:512])
            nc.sync.dma_start(x8_sb[:, :, 0, 512:1024],
                              x_src[:, :, 0, 512:1024])
            nc.sync.dma_start(w8_sb[:, :, 512:1024], w_src[:, :, 512:1024])
            nc.sync.dma_start(w8_sb[:, :, 1024:2560], w_src[:, :, 1024:2560])
            nc.sync.dma_start(x8_sb[:, :, 1, 0:512], x_src[:, :, 1, 0:512])
            nc.sync.dma_start(x8_sb[:, :, 1, 512:1024],
                              x_src[:, :, 1, 512:1024])
            nc.sync.dma_start(w8_sb[:, :, 2560:3584], w_src[:, :, 2560:3584])
            nc.sync.dma_start(w8_sb[:, :, 3584:4608], w_src[:, :, 3584:4608])
            bproj_sb = const.tile([128, C // 128], fp32, tag="bproj", name="bproj")
            nc.sync.dma_start(bproj_sb[:], bproj[:, :])
            wproj_big = const.tile([128, KC, C], f16, tag="wproj", name="wproj")
            wproj_sb = [wproj_big[:, kc] for kc in range(KC)]
            nc.sync.dma_start(
                wproj_big[:],
                wproj.rearrange("(k p) c -> p k c", k=KC),
            )
            ident_sb = const.tile([128, 128], f16, tag="ident", name="ident")
            make_identity(nc, ident_sb[:])
            # keep columns m >= l (query >= key) on the diagonal block
            trimask_sb = const.tile([128, 128], f16, tag="trimask", name="trimask")
            make_upper_triangular(nc, trimask_sb[:], val=1.0, diag=True)

            qkT_sb = [const.tile([128, T], f8, tag=f"qkT{c3}", name=f"qkT{c3}")
                      for c3 in range(2 * KC)]
            # v packed [t, 12 heads x (64 + ones col)]
            v_sb = [const.tile([128, H, D + 1], f16, tag=f"v{tt}", name=f"v{tt}")
                    for tt in range(TT)]
            yT_sb = [const.tile([128, T], f16, tag=f"yT{kc}", name=f"yT{kc}")
                     for kc in range(KC)]

            def emit_v(tt):
                # two sm-pool tiles (fast DVE-only consumers) keep v off the
                # exp-paced big ring
                psA = ps_sm.tile([128, 512], fp32, tag="sm", name="vA_ps")
                psB = ps_sm.tile([128, 512], fp32, tag="sm", name="vB_ps")
                tsl = slice(tt * 128, (tt + 1) * 128)
                terms = [(0, 0), (0, 1), (1, 0)]
                # x8l term last so v matmuls start before x8l lands
                n = 0
                for xs, ws in terms:
                    vb = WV[ws]
                    for cp in range(KC // 2):
                        kk = slice(2 * cp, 2 * cp + 2)
                        nc.tensor.matmul(
                            psA[:],
                            x8_sb[:, kk, xs, tsl],
                            w8_sb[:, kk, vb:vb + 512],
                            start=(n == 0), stop=(n == 8), perf_mode=DR,
                        )
                        nc.tensor.matmul(
                            psB[:, 0:256],
                            x8_sb[:, kk, xs, tsl],
                            w8_sb[:, kk, vb + 512:vb + 768],
                            start=(n == 0), stop=(n == 8), perf_mode=DR,
                        )
                        n += 1
                vv = v_sb[tt]
                nc.vector.tensor_scalar_mul(
                    vv[:, 0:8, 0:D],
                    psA[:].rearrange("p (h d) -> p h d", d=D),
                    1.0 / 64.0,
                )
                nc.vector.tensor_scalar_mul(
                    vv[:, 8:12, 0:D],
                    psB[:, 0:256].rearrange("p (h d) -> p h d", d=D),
                    1.0 / 64.0,
                )
                nc.vector.memset(vv[:, :, D:D + 1], 1.0)

            Identity = mybir.ActivationFunctionType.Identity

            def emit_qk_half(hp, which, tchunk, on_act=False, cold=False):
                c3 = hp if which == "q" else KC + hp
                # cold-start halves alternate the y2 pool (idle until pair
                # 0) with the dedicated bank: an effective ring-3 while DMA
                # streams
                if cold == "sm":
                    pool, tg = ps_sm, "sm"
                else:
                    pool, tg = ps_qk, "qk"
                ps = pool.tile([128, 512], fp32, tag=tg, name="qk_ps")
                sl = slice(tchunk * 512, (tchunk + 1) * 512)
                n = 0
                for ws in (0, 1):
                    wc = wcol(ws, c3)
                    for cp in range(KC // 2):
                        kk = slice(2 * cp, 2 * cp + 2)
                        nc.tensor.matmul(
                            ps[:],
                            w8_sb[:, kk, wc:wc + 128],
                            x8_sb[:, kk, 0, sl],
                            start=(n == 0), stop=(n == 5), perf_mode=DR,
                        )
                        n += 1
                if on_act:
                    nc.scalar.activation(qkT_sb[c3][:, sl], ps[:],
                                         Identity, bias=bqkv_sb[:, c3:c3 + 1])
                else:
                    nc.vector.tensor_scalar_add(
                        qkT_sb[c3][:, sl], ps[:], bqkv_sb[:, c3:c3 + 1],
                    )

            def emit_S_group(hp, segs, grp):
                qT = qkT_sb[hp]
                kT = qkT_sb[KC + hp]
                for h in (2 * hp, 2 * hp + 1):
                    poff = 64 * (h % 2)
                    S = ps_big.tile([128, 1024], fp32, tag="big", name="S")
                    span = 0
                    for j, base in grp:
                        qs = 128 * j
                        w = T - qs
                        span = base + w
                        first = base + min(512 - base % 512, w) if base < 512 \
                            else base + w
                        for a, b_ in ((base, first), (first, base + w)):
                            if b_ <= a:
                                continue
                            nc.tensor.matmul(
                                S[:, a:b_],
                                dr2(kT[poff:poff + 64, qs:qs + 128]),
                                dr2(qT[poff:poff + 64,
                                       qs + (a - base):qs + (b_ - base)]),
                                start=True, stop=True, perf_mode=DR,
                            )
                    P = pP.tile([128, 1024], f16, tag="P", name="P")
                    nc.scalar.activation(P[:, 0:span], S[:, 0:span], Exp,
                                         scale=2.0 ** -16)
                    for j, base in grp:
                        nc.gpsimd.tensor_mul(P[:, base:base + 128],
                                             P[:, base:base + 128],
                                             trimask_sb[:])
                        segs[h][j] = (P, base)

            def emit_S_j4_pair(hp, segs):
                # both heads' j4 block (512 cols each) share one PSUM tile
                # and one exp: halves the ACT op overhead for this group
                qT = qkT_sb[hp]
                kT = qkT_sb[KC + hp]
                S = ps_big.tile([128, 1024], fp32, tag="big", name="S")
                for idx, h in enumerate((2 * hp, 2 * hp + 1)):
                    poff = 64 * (h % 2)
                    nc.tensor.matmul(
                        S[:, 512 * idx:512 * idx + 512],
                        dr2(kT[poff:poff + 64, 512:640]),
                        dr2(qT[poff:poff + 64, 512:1024]),
                        start=True, stop=True, perf_mode=DR,
                    )
                P = pP.tile([128, 1024], f16, tag="P", name="P")
                nc.scalar.activation(P[:], S[:], Exp, scale=2.0 ** -16)
                for idx, h in enumerate((2 * hp, 2 * hp + 1)):
                    base = 512 * idx
                    nc.gpsimd.tensor_mul(P[:, base:base + 128],
                                         P[:, base:base + 128],
                                         trimask_sb[:])
                    segs[h][4] = (P, base)

            def emit_AV_half(hp, segs, yns, i, y2, half):
                pair = (2 * hp, 2 * hp + 1)
                b0 = 256 * half
                for idx, h in enumerate(pair):
                    c0 = b0 + 128 * idx
                    for j in range(i + 1):
                        P, base = segs[h][j]
                        off = base + 128 * (i - j)
                        nc.tensor.matmul(
                            y2[:, c0:c0 + D + 1],
                            P[:, off:off + 128],
                            v_sb[j][:, h, :],
                            start=(j == 0), stop=(j == i),
                        )
                recip = small.tile([128, 2], fp32, tag="recip", name="recip")
                nc.vector.reciprocal(
                    recip[:],
                    y2[:].rearrange("p (g c) -> p g c", c=128)[:, 2 * half:2 * half + 2, D],
                )
                # both heads' normalized y packed [128, 128] -> one transpose
                yn = small.tile([128, 2 * D], f16, tag="yn", name="yn",
                                bufs=10)
                for idx, h in enumerate(pair):
                    c0 = b0 + 128 * idx
                    nc.vector.tensor_scalar_mul(yn[:, idx * D:(idx + 1) * D],
                                                y2[:, c0:c0 + D],
                                                recip[:, idx:idx + 1])
                yns.append((i, yn))

            def emit_yT(hp, trs):
                nc.vector.tensor_scalar_add(
                    yT_sb[hp][:],
                    trs[:],
                    bqkv_sb[:, 2 * KC + hp:2 * KC + hp + 1],
                )

            def new_segs():
                return {h: {} for h in range(H)}

            # ---- cold start: qk(0) first, then S(0) groups (j0-first so
            # AV(0,0) unblocks early) with 1:1 v backfill ----
            segs = {0: new_segs(), 1: new_segs()}
            coldseq = ["sm"] * 8
            emit_qk_half(0, "q", 0, cold=coldseq[0])
            emit_qk_half(0, "k", 0, cold=coldseq[1])
            emit_qk_half(0, "q", 1, cold=coldseq[2])
            emit_S_group(0, segs[0], GROUPS[4])
            emit_qk_half(0, "k", 1, cold=coldseq[3])
            for n, (which, tchunk) in enumerate(
                    (("q", 0), ("q", 1), ("k", 0), ("k", 1))):
                emit_qk_half(1, which, tchunk, cold=coldseq[4 + n])
                if n < 3:
                    emit_S_group(0, segs[0], GROUPS[3 - n])
                else:
                    emit_S_j4_pair(0, segs[0])
            # v tiles before S(1): v's consumers (DVE copies) release big-
            # pool slots fast, while S tiles are released by ACT exps — v
            # first keeps the ring exp-decoupled and AV(0) unblocks early.
            # S(1)'s exps then keep ACT busy into pair 0 (depth-2 S/exp
            # pipeline: S(hp+2) is emitted during pair hp).
            emit_S_group(1, segs[1], GROUPS[4])
            emit_S_group(1, segs[1], GROUPS[3])
            emit_S_group(1, segs[1], GROUPS[2])
            emit_S_group(1, segs[1], GROUPS[1])
            emit_S_j4_pair(1, segs[1])
            for n in range(6):
                emit_v(n)

            # ---- pipelined cycles ----
            # per cycle: 8 AV i-steps; qk(hp+1) halves at steps 0,1,3; S(hp+1)
            # groups j0-first at steps 2,4,5,6,7 (j0 consumed first next cycle).
            qk_order = [("q", 0), ("q", 1), ("k", 0), ("k", 1)]
            def emit_transpose_slice(trs, yns, sl):
                for i, yn in yns[sl]:
                    nc.tensor.transpose(trs[:, 128 * i:128 * (i + 1)],
                                        yn[:], ident_sb[:])

            def emit_proj_mms(ps, cc, a, b_, kcs, stop_kc):
                for kc in kcs:
                    nc.tensor.matmul(
                        ps[:, a:b_],
                        wproj_sb[kc][:, cc * 128:(cc + 1) * 128],
                        yT_sb[kc][:, a:b_],
                        start=(kc == 0), stop=(kc == stop_kc),
                    )

            # staged projection partials: per (cc, half) a [128,512] unit
            # on the sm/qk rotation (the big pool is held by S-tile exps
            # until the train drains), bias-copied to SBUF on DVE/Pool (ACT
            # is exp-saturated through pair 4)
            proj_part = {}

            def unit_proj_half(cc, half, n):
                a, b_ = 512 * half, 512 * (half + 1)

                def f():
                    pool, tg = (ps_sm, "sm") if n % 3 != 2 else (ps_qk, "qk")
                    ps = pool.tile([128, 512], fp32, tag=tg, name="o_part_ps")
                    for kc in range(3):
                        nc.tensor.matmul(
                            ps[:],
                            wproj_sb[kc][:, cc * 128:(cc + 1) * 128],
                            yT_sb[kc][:, a:b_],
                            start=(kc == 0), stop=(kc == 2),
                        )
                    if cc not in proj_part:
                        proj_part[cc] = osb.tile([128, T], f16, tag="o_part",
                                                 name="o_part", bufs=6)
                    nc.vector.tensor_scalar_add(
                        proj_part[cc][:, a:b_], ps[:],
                        bproj_sb[:, cc:cc + 1])
                return f

            proj_units = [unit_proj_half(cc, half, 2 * cc + half)
                          for cc in range(KC) for half in (0, 1)]

            prev_yns = None
            for hp in range(NPAIR):
                nxt = hp + 1 < NPAIR
                last = not nxt
                if hp + 2 < NPAIR:
                    segs[hp + 2] = new_segs()
                y2 = None
                yns = []
                trs = ps_tr.tile([128, 1024], f16, tag="tr", name="tr") \
                    if prev_yns is not None else None
                for i in range(TT):
                    if i % 2 == 0:
                        y2 = ps_sm.tile([128, 512], fp32, tag="sm",
                                        name="y2")
                    emit_AV_half(hp, segs[hp], yns, i, y2, i % 2)
                    if hp == 0 and i in (3, 5):
                        emit_v(6 if i == 3 else 7)
                    if prev_yns is not None:
                        emit_transpose_slice(trs, prev_yns,
                                             slice(i, i + 1))
                    if hp + 2 < NPAIR and i <= 3:
                        emit_qk_half(hp + 2, *qk_order[i])
                    if hp == 0:
                        gidx1 = {0: 4, 1: 3, 2: 2, 3: 1}.get(i)
                        if gidx1 is not None:
                            emit_S_group(1, segs[1], GROUPS[gidx1])
                        elif i == 4:
                            emit_S_j4_pair(1, segs[1])
                        gidx2 = {4: 4, 5: 3, 6: 2, 7: 1}.get(i)
                        if gidx2 is not None:
                            emit_S_group(2, segs[2], GROUPS[gidx2])
                    elif hp + 2 < NPAIR:
                        gidx = {3: 4, 4: 3, 5: 2, 6: 1}.get(i)
                        if gidx is not None:
                            emit_S_group(hp + 2, segs[hp + 2], GROUPS[gidx])
                        elif i == 7:
                            emit_S_j4_pair(hp + 2, segs[hp + 2])
                    elif hp == NPAIR - 2 and proj_units:
                        # pair 4 is qk/S-free: drain projection partials
                        # (kc 0-2; yT[0..2] final after pair 3) into the
                        # idle big pool
                        for u in (proj_units.pop(0) for _ in
                                  range(min(2, len(proj_units)))):
                            u()
                if hp == 0:
                    emit_S_j4_pair(2, segs[2])
                if prev_yns is not None:
                    emit_yT(hp - 1, trs)
                prev_yns = yns
                segs.pop(hp)
                if last:
                    while proj_units:
                        proj_units.pop(0)()
            trs = ps_tr.tile([128, 1024], f16, tag="tr", name="tr")
            hp5 = NPAIR - 1
            emit_transpose_slice(trs, prev_yns, slice(0, 4))
            nc.vector.tensor_scalar_add(
                yT_sb[hp5][:, 0:512], trs[:, 0:512],
                bqkv_sb[:, 2 * KC + hp5:2 * KC + hp5 + 1])
            emit_transpose_slice(trs, prev_yns, slice(4, 8))
            nc.vector.tensor_scalar_add(
                yT_sb[hp5][:, 512:1024], trs[:, 512:1024],
                bqkv_sb[:, 2 * KC + hp5:2 * KC + hp5 + 1])

            # ---- projection endgame: kc3-5 accumulation + DVE merge with
            # the staged bias-carrying kc0-2 partials ----
            for cc in range(KC):
                ps = ps_big.tile([128, 1024], fp32, tag="big", name="o_ps")
                for a, b_ in ((0, 512), (512, 1024)):
                    for kc in (3, 4, 5):
                        nc.tensor.matmul(
                            ps[:, a:b_],
                            wproj_sb[kc][:, cc * 128:(cc + 1) * 128],
                            yT_sb[kc][:, a:b_],
                            start=(kc == 3), stop=(kc == 5),
                        )
                o = osb.tile([128, T], f16, tag="o_sb", name="o_sb")
                nc.vector.tensor_add(o[:], ps[:], proj_part[cc][:])
                nc.sync.dma_start(out[cc * 128:(cc + 1) * 128, :], o[:])

    nc.compile()
    return nc


def _split_f8(a):
    hi = a.astype(_F8)
    lo = (a - hi.astype(np.float32)).astype(_F8)
    return np.ascontiguousarray(hi), np.ascontiguousarray(lo)


def _prep_inputs(x, w_qkv, b_qkv, w_proj, b_proj):
    # w scaled by 64 so fp8e4m3 quantization of the ~0.02-scale weights (and
    # their residuals) stays in the normal range; q/k biases scale to match
    # (exp scale folds the 64^2 back out); v descales at the on-chip copy.
    w64 = (w_qkv.astype(np.float32)) * 64.0
    wqkv_h, wqkv_l = _split_f8(w64)
    wqkv_8 = np.ascontiguousarray(np.concatenate([
        wqkv_h[:, 0:256], wqkv_l[:, 0:256],          # q01 h|l
        wqkv_h[:, C:C + 256], wqkv_l[:, C:C + 256],  # k01 h|l
        wqkv_h[:, 2 * C:], wqkv_l[:, 2 * C:],        # v h|l
        wqkv_h[:, 256:C], wqkv_l[:, 256:C],          # q25 h|l
        wqkv_h[:, C + 256:2 * C], wqkv_l[:, C + 256:2 * C],  # k25 h|l
    ], axis=1))
    wproj_f = np.ascontiguousarray(w_proj.astype(_F16))
    b_sc = b_qkv.astype(np.float32).copy()
    b_sc[:2 * C] *= 64.0
    bqkv_pc = np.ascontiguousarray(b_sc.reshape(C3 // 128, 128).T)
    bproj_pc = np.ascontiguousarray(
        b_proj.astype(np.float32).reshape(C // 128, 128).T)
    in_maps = []
    for b in range(B):
        xTb = np.ascontiguousarray(x[b].astype(np.float32).T)
        xh, xl = _split_f8(xTb)
        in_maps.append({
            "xT8": np.ascontiguousarray(np.stack([xh, xl], axis=1)),
            "wqkv8": wqkv_8,
            "wproj": wproj_f,
            "bqkv": bqkv_pc,
            "bproj": bproj_pc,
        })
    return in_maps


def _run(inputs, trace=False):
    from concourse.bass_utils import run_bass_kernel_spmd

    if "nc" not in _compiled:
        _compiled["nc"] = _build()
    nc = _compiled["nc"]
    in_maps = _prep_inputs(inputs["x"], inputs["w_qkv"], inputs["b_qkv"],
                           inputs["w_proj"], inputs["b_proj"])
    res = run_bass_kernel_spmd(nc, in_maps, list(range(B)), trace=trace)
    outs = np.stack([np.asarray(res.results[b]["out"]).T for b in range(B)])
    return outs.astype(np.float32), res


def kernel(x, w_qkv, b_qkv, w_proj, b_proj):
    outs, _ = _run(dict(x=x, w_qkv=w_qkv, b_qkv=b_qkv,
                        w_proj=w_proj, b_proj=b_proj))
    return outs
